# revision 1
# baseline (speedup 1.0000x reference)
"""Canny edge detector on 8 Trainium2 NeuronCores (Bass/Tile).

Sharding: row slabs. Core i owns output rows [118*i, 118*(i+1)) of ALL 8
images. (The reference's flat gather at B=8 cross-wires images inside NMS:
sel_pos(b,h,w) = dirconv_b(gm_{idx(b,h,w)})(h,w), so every output pixel needs
all 8 images' gradient-magnitude maps at its rows -> shard by rows, not by
image.) The leftover band (rows 944..1023) is computed per-image on the
owning core and the gm maps are exchanged through DRAM collectives
(AllGather for plain maps, AllToAll for reader-shift-specific maps).

All compute-engine APs must start at partition 0 (HW constraint), so row
re-alignment between pipeline stages is done with SBUF->SBUF DMAs.
"""

import os

# Tile's subtile dependency tracking emits >1 embedded sync-wait on
# S2S2D2_STT instructions, which the ISA encoding cannot hold ("Too many
# sync wait commands" in codegen). With whole-tile deps the wait-absorbing
# guard ops keep every STT at <=1 embedded wait.
os.environ.setdefault("BY_DEFAULT_DISABLE_SUBTILE_DEPS", "1")

import numpy as np

H = 1024
W = 1024
B = 8
NC = 8
SLAB = 118                    # main-slab output rows per core
B8_START = SLAB * NC          # 944
B8_ROWS = H - B8_START        # 80
LOW_T, HIGH_T = 2.5, 5.0
T22SQ = float(np.float32(np.tan(np.pi / 8.0)) ** 2)

# direction index -> (dr, dc) neighbor offset of dir_f channel d
DELTAS = {0: (0, 1), 1: (1, 1), 2: (1, 0), 3: (1, -1),
          4: (0, -1), 5: (-1, -1), 6: (-1, 0), 7: (-1, 1)}


def _gauss5():
    n = np.arange(5, dtype=np.float32) - 2.0
    return np.exp(-0.5 * n * n).astype(np.float32)


def _band(n_in, n_out, offset, taps):
    """M[k, m] = taps[k - m - offset] for k-m-offset in range(len(taps))."""
    m_ = np.zeros((n_in, n_out), np.float32)
    for mm in range(n_out):
        for t, w in enumerate(taps):
            k = mm + offset + t
            if 0 <= k < n_in:
                m_[k, mm] = w
    return m_


def _const_mats(core):
    g = _gauss5()
    g0 = float(g[0])
    mats = {}
    # main slab: x/hb tile row k <-> img row a+k, a = 118i-5
    # bl row m <-> img a+2+m (124 rows); BV[k,m] = g0*g[k-m]
    mats["BV"] = _band(128, 124, 0, (g0 * g).tolist())
    # gx/gy row m <-> img a+3+m = 118i-2+m (122 rows); bl k <-> a+2+k
    b121 = _band(124, 122, 0, [1.0, 2.0, 1.0])
    b10m1 = _band(124, 122, 0, [1.0, 0.0, -1.0])
    if core == 0:  # img rows -2,-1 must yield gm=0 (zero-pad semantics)
        b121[:, 0:2] = 0.0
        b10m1[:, 0:2] = 0.0
    mats["B121"] = b121
    mats["B121N"] = -b121
    mats["B10M1"] = b10m1
    mats["B10M1X2"] = 2.0 * b10m1
    # strong row k (base 0) <-> thin img row 118i-1+k
    # mp row p <-> img 118i-1+p (col 0 dummy); needs strong k = p-1,p,p+1
    bones = _band(120, 119, -1, [1.0, 1.0, 1.0])
    bones[:, 0] = 0.0
    if core == 0:
        bones[:, 1] = 0.0  # border row 0
    mats["BONES"] = bones
    # B8 block: x8 row k <-> img 936+k; bl8 row m <-> img 938+m (86 rows)
    mats["BV8"] = _band(88, 86, 0, (g0 * g).tolist())
    # gx8 row m <-> img 940+m (84 rows); bl8 k <-> 938+k: band k-m in {1,2,3}
    b121_8 = _band(86, 84, 1, [1.0, 2.0, 1.0])
    b10m1_8 = _band(86, 84, 1, [1.0, 0.0, -1.0])
    mats["B121_8"] = b121_8
    mats["B121N_8"] = -b121_8
    mats["B10M1_8"] = b10m1_8
    mats["B10M1X2_8"] = 2.0 * b10m1_8
    # strong8 row k (base 0) <-> img 943+k; mp8 row p <-> img 943+p
    # (col 0 dummy); needs strong8 k = p-1,p,p+1
    bones8 = _band(81, 81, -1, [1.0, 1.0, 1.0])
    bones8[:, 0] = 0.0
    bones8[:, 80] = 0.0  # border row 1023
    mats["BONES8"] = bones8
    return {k: np.ascontiguousarray(v, np.float32) for k, v in mats.items()}


MAT_SPECS = {
    "BV": [128, 124], "B121": [124, 122], "B121N": [124, 122],
    "B10M1": [124, 122], "B10M1X2": [124, 122], "BONES": [120, 119],
    "BV8": [88, 86], "B121_8": [86, 84], "B121N_8": [86, 84],
    "B10M1_8": [86, 84], "B10M1X2_8": [86, 84], "BONES8": [81, 81],
}

_CACHE = {}


def _build_program():
    if "nc" in _CACHE:
        return _CACHE["nc"]
    import concourse.bass as bass
    import concourse.mybir as mybir
    from concourse.tile import TileContext

    f32 = mybir.dt.float32
    bf16 = mybir.dt.bfloat16
    u8 = mybir.dt.uint8
    Alu = mybir.AluOpType

    g = _gauss5()
    r01 = float(g[0] / g[1])
    r12 = float(g[1] / g[2])
    r21 = float(g[2] / g[1])
    r10 = float(g[1] / g[0])

    nc = bass.Bass()

    def guard(out_ap, in0_ap, in1_ap):
        # Obsolete: _legalize_waits() NoOp-splits any multi-wait instruction
        # after scheduling, which is cheaper than extra DVE data ops.
        pass

    def fence(t):
        pass

    xm = nc.declare_dram_parameter("xm", [B * 3, 128, W], f32, isOutput=False)
    x8 = nc.declare_dram_parameter("x8", [3, 88, W], f32, isOutput=False)
    mat_d = {k: nc.declare_dram_parameter(k, v, f32, isOutput=False)
             for k, v in MAT_SPECS.items()}
    outm = nc.declare_dram_parameter("outm", [B, SLAB, W], f32, isOutput=True)
    out8 = nc.declare_dram_parameter("out8", [B8_ROWS, W], f32, isOutput=True)

    with TileContext(nc) as tc:
        with (
            tc.tile_pool(name="consts", bufs=1) as cpool,
            tc.tile_pool(name="gmp", bufs=1) as gmpool,
            tc.tile_pool(name="msk", bufs=1) as mskpool,
            tc.tile_pool(name="dram", bufs=1, space="DRAM") as dpool,
        ):
            mt = {}
            for name, shp in MAT_SPECS.items():
                t = cpool.tile(shp, f32, tag=name)
                nc.sync.dma_start(out=t[:], in_=mat_d[name][:])
                mt[name] = t

            gm_tiles = []
            masks = []
            # =========== conv phase (own scoped pools) =====================
            with (
                tc.tile_pool(name="xin", bufs=3) as xpool,
                tc.tile_pool(name="hbt", bufs=2) as hbpool,
                tc.tile_pool(name="bls", bufs=2) as blspool,
                tc.tile_pool(name="sq", bufs=2) as sqpool,
                tc.tile_pool(name="gsum", bufs=2) as gsumpool,
                tc.tile_pool(name="mskt", bufs=2) as msktpool,
                tc.tile_pool(name="psA", bufs=2, space="PSUM") as psA,
                tc.tile_pool(name="psB", bufs=1, space="PSUM") as psB,
            ):
                def conv_pipeline(xt, n_in, bv, b121, b121n, b10m1, b10m1x2,
                                  gm_acc, gxs, gys, c, n_bl, n_gxy,
                                  pe_hblur=False):
                    bl = psA.tile([n_bl, W], f32, tag="bl")
                    if pe_hblur:
                        # full 2D blur as 5 shifted-column accumulated
                        # streams: bl = sum_h (g_h * band(g)) @ x<<(h-2)
                        lhs5 = [bv, mt["BVG1"], mt["BVG2"], mt["BVG1"], bv]
                        for lo in (0, 512):
                            for h in range(5):
                                nc.tensor.matmul(
                                    out=bl[:, lo:lo + 512],
                                    lhsT=lhs5[h][0:n_in, 0:n_bl],
                                    rhs=xt[:, h + lo:h + lo + 512],
                                    start=(h == 0), stop=(h == 4))
                    else:
                        # H-blur (Horner, 4 fused ops) -> h2 [n_in, W]
                        h1 = hbpool.tile([n_in, W], f32, tag="h1")
                        h2 = hbpool.tile([n_in, W], f32, tag="h2")
                        guard(h1, xt, h2)
                        guard(h2, xt, h1)
                        nc.vector.scalar_tensor_tensor(
                            out=h1[:], in0=xt[:, 0:W], scalar=r01,
                            in1=xt[:, 1:W + 1], op0=Alu.mult, op1=Alu.add)
                        nc.vector.scalar_tensor_tensor(
                            out=h2[:], in0=h1[:], scalar=r12,
                            in1=xt[:, 2:W + 2], op0=Alu.mult, op1=Alu.add)
                        nc.vector.scalar_tensor_tensor(
                            out=h1[:], in0=h2[:], scalar=r21,
                            in1=xt[:, 3:W + 3], op0=Alu.mult, op1=Alu.add)
                        nc.vector.scalar_tensor_tensor(
                            out=h2[:], in0=h1[:], scalar=r10,
                            in1=xt[:, 4:W + 4], op0=Alu.mult, op1=Alu.add)
                        for lo in (0, 512):
                            nc.tensor.matmul(out=bl[:, lo:lo + 512],
                                             lhsT=bv[0:n_in, 0:n_bl],
                                             rhs=h2[:, lo:lo + 512],
                                             start=True, stop=True)
                    # copy to SBUF with 1-col zero margins
                    blt = blspool.tile([n_bl, W + 2], f32, tag="bls")
                    fence(blt)
                    nc.vector.memset(blt[:, 0:1], 0.0)
                    nc.vector.memset(blt[:, W + 1:W + 2], 0.0)
                    nc.scalar.copy(out=blt[:, 1:W + 1], in_=bl[:])
                    blm = blt[:, 0:W]
                    blc = blt[:, 1:W + 1]
                    blp = blt[:, 2:W + 2]
                    # sobel on PE: gx = B121@blm - B121@blp
                    #              gy = B10M1@(blp+blm) + 2*B10M1@blc
                    gx = psB.tile([n_gxy, W], f32, tag="gx")
                    gy = psB.tile([n_gxy, W], f32, tag="gy")
                    for lo in (0, 512):
                        nc.tensor.matmul(out=gx[:, lo:lo + 512],
                                         lhsT=b121[0:n_bl, 0:n_gxy],
                                         rhs=blm[:, lo:lo + 512],
                                         start=True, stop=False)
                        nc.tensor.matmul(out=gx[:, lo:lo + 512],
                                         lhsT=b121n[0:n_bl, 0:n_gxy],
                                         rhs=blp[:, lo:lo + 512],
                                         start=False, stop=True)
                        nc.tensor.matmul(out=gy[:, lo:lo + 512],
                                         lhsT=b10m1[0:n_bl, 0:n_gxy],
                                         rhs=blp[:, lo:lo + 512],
                                         start=True, stop=False)
                        nc.tensor.matmul(out=gy[:, lo:lo + 512],
                                         lhsT=b10m1x2[0:n_bl, 0:n_gxy],
                                         rhs=blc[:, lo:lo + 512],
                                         start=False, stop=False)
                        nc.tensor.matmul(out=gy[:, lo:lo + 512],
                                         lhsT=b10m1[0:n_bl, 0:n_gxy],
                                         rhs=blm[:, lo:lo + 512],
                                         start=False, stop=True)
                    # magnitude
                    sqx = sqpool.tile([n_gxy, W], f32, tag="sqx")
                    sqy = sqpool.tile([n_gxy, W], f32, tag="sqy")
                    nc.scalar.square(out=sqx[:], in_=gx[:])
                    nc.scalar.square(out=sqy[:], in_=gy[:])
                    m2 = sqpool.tile([n_gxy, W], f32, tag="m2")
                    nc.gpsimd.tensor_tensor(out=m2[:], in0=sqx[:], in1=sqy[:],
                                            op=Alu.add)
                    if c == 0:
                        nc.scalar.sqrt(out=gm_acc[0:n_gxy, 1:W + 1], in_=m2[:])
                    else:
                        magt = sqpool.tile([n_gxy, W], f32, tag="magt")
                        nc.scalar.sqrt(out=magt[:], in_=m2[:])
                        nc.gpsimd.tensor_tensor(
                            out=gm_acc[0:n_gxy, 1:W + 1],
                            in0=gm_acc[0:n_gxy, 1:W + 1],
                            in1=magt[:], op=Alu.add)
                    # gxs/gys accumulation (full range, base partition 0)
                    if c == 0:
                        nc.scalar.copy(out=gxs[0:n_gxy, :], in_=gx[:])
                        nc.scalar.copy(out=gys[0:n_gxy, :], in_=gy[:])
                    else:
                        nc.vector.tensor_tensor(out=gxs[0:n_gxy, :],
                                                in0=gxs[0:n_gxy, :],
                                                in1=gx[:], op=Alu.add)
                        nc.vector.tensor_tensor(out=gys[0:n_gxy, :],
                                                in0=gys[0:n_gxy, :],
                                                in1=gy[:], op=Alu.add)

                def make_masks(gxs, gys, n, shift, n_thin, j):
                    """u8 masks computed at conv frame [0:n], DMA-shifted down
                    by `shift` rows into persistent thin-frame tiles."""
                    a2 = sqpool.tile([n, W], f32, tag="sqx")
                    b2 = sqpool.tile([n, W], f32, tag="sqy")
                    nc.scalar.square(out=a2[:, :], in_=gxs[0:n, :])
                    nc.scalar.square(out=b2[:, :], in_=gys[0:n, :])
                    tmp = [msktpool.tile([n, W], u8, tag=t, name=t)
                           for t in ("tc0", "tc2", "tsm")]
                    guard(tmp[0], a2, b2)
                    guard(tmp[1], a2, b2)
                    nc.vector.scalar_tensor_tensor(
                        out=tmp[0][:], in0=a2[:], scalar=T22SQ,
                        in1=b2[:], op0=Alu.mult, op1=Alu.is_gt)
                    nc.vector.scalar_tensor_tensor(
                        out=tmp[1][:], in0=b2[:], scalar=T22SQ,
                        in1=a2[:], op0=Alu.mult, op1=Alu.is_gt)
                    ab = sqpool.tile([n, W], f32, tag="m2")
                    nc.gpsimd.tensor_tensor(out=ab[:], in0=gxs[0:n, :],
                                            in1=gys[0:n, :], op=Alu.mult)
                    guard(tmp[2], ab, ab)
                    nc.vector.tensor_scalar(out=tmp[2][:], in0=ab[:],
                                            scalar1=0.0, scalar2=None,
                                            op0=Alu.is_ge)
                    out = []
                    for t, tag in zip(tmp, ("c0", "c2", "sm")):
                        p = mskpool.tile([n_thin, W], u8, tag=f"{tag}_{j}")
                        fence(p)
                        nc.sync.dma_start(out=p[:],
                                          in_=t[shift:shift + n_thin, :])
                        out.append(p)
                    return out

                # main slab: 8 images x 3 channels
                for j in range(B):
                    gm_j = gmpool.tile([122, W + 2], f32, tag=f"gm{j}")
                    nc.vector.memset(gm_j[:, 0:1], 0.0)
                    nc.vector.memset(gm_j[:, W + 1:W + 2], 0.0)
                    gxs = gsumpool.tile([122, W], f32, tag="gxs")
                    gys = gsumpool.tile([122, W], f32, tag="gys")
                    for c in range(3):
                        xt = xpool.tile([128, W + 4], f32, tag="x")
                        fence(xt)
                        nc.vector.memset(xt[:, 0:2], 0.0)
                        nc.vector.memset(xt[:, W + 2:W + 4], 0.0)
                        nc.sync.dma_start(out=xt[:, 2:W + 2], in_=xm[3 * j + c])
                        conv_pipeline(xt, 128, mt["BV"], mt["B121"],
                                      mt["B121N"], mt["B10M1"], mt["B10M1X2"],
                                      gm_j, gxs, gys, c, 124, 122)
                    gm_tiles.append(gm_j)
                    # thin frame = conv rows 1..120 -> shift 1, 120 rows
                    masks.append(make_masks(gxs, gys, 122, 1, 120, j))

                # B8 block (own image); gm8 row p <-> img 940+p, row 84 = 0
                gm8 = gmpool.tile([85, W + 2], f32, tag="gm8self")
                nc.vector.memset(gm8[:], 0.0)
                gxs8 = gsumpool.tile([84, W], f32, tag="gxs")
                gys8 = gsumpool.tile([84, W], f32, tag="gys")
                for c in range(3):
                    xt = xpool.tile([88, W + 4], f32, tag="x")
                    fence(xt)
                    nc.vector.memset(xt[:, 0:2], 0.0)
                    nc.vector.memset(xt[:, W + 2:W + 4], 0.0)
                    nc.sync.dma_start(out=xt[:, 2:W + 2], in_=x8[c])
                    conv_pipeline(xt, 88, mt["BV8"], mt["B121_8"],
                                  mt["B121N_8"], mt["B10M1_8"],
                                  mt["B10M1X2_8"], gm8, gxs8, gys8, c, 86, 84)
                # thin8 frame = conv rows 3..83 -> shift 3, 81 rows
                m8 = make_masks(gxs8, gys8, 84, 3, 81, 8)

            # =========== B8 gm exchange ===================================
            ag_in = dpool.tile([81, W], f32, tag="ag_in")
            ag_out = dpool.tile([B * 81, W], f32, tag="ag_out")
            fence(gm8)
            nc.sync.dma_start(out=ag_in[:], in_=gm8[3:84, 1:W + 1])
            nc.gpsimd.collective_compute(
                "AllGather", Alu.bypass, replica_groups=[list(range(NC))],
                ins=[ag_in.opt()], outs=[ag_out.opt()])
            a2a_in = dpool.tile([B * 81, W], f32, tag="a2a_in")
            a2a_out = dpool.tile([B * 81, W], f32, tag="a2a_out")
            for b in range(B):
                dr, dc = DELTAS[b]
                nc.sync.dma_start(
                    out=a2a_in[81 * b:81 * (b + 1), :],
                    in_=gm8[3 + dr:84 + dr, 1 + dc:W + 1 + dc])
            nc.gpsimd.collective_compute(
                "AllToAll", Alu.bypass, replica_groups=[list(range(NC))],
                ins=[a2a_in.opt()], outs=[a2a_out.opt()])

            # =========== NMS phase (own scoped pools) ======================
            # thin frame: row p (base 0) <-> img row 118i-1+p, 120 rows.
            with (
                tc.tile_pool(name="ce", bufs=1) as cepool,
                tc.tile_pool(name="shp", bufs=1) as shpool,
                tc.tile_pool(name="cmap", bufs=2) as cpool2,
                tc.tile_pool(name="g8p", bufs=2) as g8pool,
                tc.tile_pool(name="pmap", bufs=1) as ppool,
                tc.tile_pool(name="nmst", bufs=1) as npool,
                tc.tile_pool(name="outp", bufs=2) as opool,
                tc.tile_pool(name="psC", bufs=2, space="PSUM") as psC,
            ):
                # center-aligned copies of gm (thin frame)
                ce = []
                for j in range(B):
                    fence(gm_tiles[j])
                    t = cepool.tile([120, W + 2], f32, tag=f"ce{j}")
                    nc.sync.dma_start(out=t[:], in_=gm_tiles[j][1:121, :])
                    ce.append(t)

                def build_shift(drow):
                    tiles = []
                    for j in range(B):
                        t = shpool.tile([120, W + 2], f32, tag=f"sh{j}")
                        fence(t)
                        if drow == 1:
                            nc.sync.dma_start(out=t[:],
                                              in_=gm_tiles[j][2:122, :])
                        else:
                            nc.sync.dma_start(out=t[:],
                                              in_=gm_tiles[j][0:120, :])
                        tiles.append(t)
                    return tiles

                def nms_core(b_masks, gm_b, get_in0, get_in1, n_thin,
                             bones, n_mp, out_lo, out_dram, n_out):
                    """Shared NMS tail; all tiles base partition 0."""
                    c0, c2, sm = b_masks
                    P = []
                    for k in range(4):
                        Cs = []
                        for j in (k, k + 4):
                            cj = cpool2.tile([n_thin, W], bf16, tag="c")
                            nc.vector.tensor_tensor(out=cj[:], in0=get_in0(j),
                                                    in1=get_in1(j),
                                                    op=Alu.is_gt)
                            Cs.append(cj)
                        tag = "psel" if k == 3 else f"p{k}"
                        bufs_k = 2 if k == 3 else None
                        pk = ppool.tile([n_thin, W], bf16, tag=tag,
                                        bufs=bufs_k)
                        nc.vector.tensor_tensor(out=pk[:], in0=Cs[0][:],
                                                in1=Cs[1][:],
                                                op=Alu.logical_and)
                        P.append(pk)
                    psel = P[3]
                    nc.vector.copy_predicated(out=psel[:], mask=sm[:],
                                              data=P[1][:])
                    nc.vector.copy_predicated(out=psel[:], mask=c0[:],
                                              data=P[0][:])
                    nc.vector.copy_predicated(out=psel[:], mask=c2[:],
                                              data=P[2][:])
                    strong = npool.tile([n_thin, W + 2], f32, tag="strong", bufs=2)
                    fence(strong)
                    nc.vector.memset(strong[:, 0:1], 0.0)
                    nc.vector.memset(strong[:, W + 1:W + 2], 0.0)
                    guard(strong, gm_b, psel)
                    nc.vector.scalar_tensor_tensor(
                        out=strong[:, 1:W + 1], in0=gm_b, scalar=HIGH_T,
                        in1=psel[:], op0=Alu.is_gt, op1=Alu.logical_and)
                    q = npool.tile([n_thin, W], f32, tag="q")
                    guard(q, gm_b, psel)
                    nc.vector.scalar_tensor_tensor(
                        out=q[:], in0=gm_b, scalar=LOW_T, in1=psel[:],
                        op0=Alu.is_ge, op1=Alu.logical_and)
                    mh = npool.tile([n_thin, W], f32, tag="mh")
                    nc.gpsimd.tensor_tensor(out=mh[:], in0=strong[:, 0:W],
                                            in1=strong[:, 2:W + 2],
                                            op=Alu.add)
                    nc.gpsimd.tensor_tensor(out=mh[:], in0=mh[:],
                                            in1=strong[:, 1:W + 1],
                                            op=Alu.add)
                    mp = psC.tile([n_mp, W], f32, tag="mp")
                    for lo2 in (0, 512):
                        nc.tensor.matmul(out=mp[:, lo2:lo2 + 512],
                                         lhsT=bones[0:n_thin, 0:n_mp],
                                         rhs=mh[:, lo2:lo2 + 512],
                                         start=True, stop=True)
                    ot = opool.tile([n_mp, W], f32, tag="ot")
                    guard(ot, mp, q)
                    nc.vector.scalar_tensor_tensor(
                        out=ot[:], in0=mp[:], scalar=0.5, in1=q[0:n_mp, :],
                        op0=Alu.is_ge, op1=Alu.logical_and)
                    nc.vector.memset(ot[:, 0:1], 0.0)
                    nc.vector.memset(ot[:, W - 1:W], 0.0)
                    nc.sync.dma_start(out=out_dram,
                                      in_=ot[out_lo:out_lo + n_out, :])

                def nms_b(b, shifted):
                    dr, dc = DELTAS[b]

                    def in0(j):
                        return ce[j][:, 1:W + 1]

                    def in1(j):
                        src = ce[j] if dr == 0 else shifted[j]
                        return src[:, 1 + dc:W + 1 + dc]

                    nms_core(masks[b], ce[b][:, 1:W + 1], in0, in1, 120,
                             mt["BONES"], 119, 1, outm[b], SLAB)

                for b in (0, 4):
                    nms_b(b, None)
                dn = build_shift(1)
                for b in (1, 2, 3):
                    nms_b(b, dn)
                up = build_shift(-1)
                for b in (5, 6, 7):
                    nms_b(b, up)

                # B8: own image only; operands pre-shifted via AllToAll.
                # thin8 frame: row p (base 0) <-> img 943+p, 81 rows.
                ce8 = g8pool.tile([81, W], f32, tag="ce8", bufs=1)
                nc.sync.dma_start(out=ce8[:], in_=gm8[3:84, 1:W + 1])

                def load8(dram_src, tag):
                    def get(j):
                        t = g8pool.tile([81, W], f32, tag=tag)
                        fence(t)
                        nc.sync.dma_start(
                            out=t[:], in_=dram_src[81 * j:81 * (j + 1), :])
                        return t[:]
                    return get

                nms_core(m8, ce8[:], load8(ag_out, "gp8"),
                         load8(a2a_out, "gs8"), 81,
                         mt["BONES8"], 81, 1, out8[:], B8_ROWS)

    _legalize_waits(nc)
    _CACHE["nc"] = nc
    return nc


def _legalize_waits(nc):
    """Several ISA encodings (S2S2D2_STT, HWDGE DMACopy, ...) hold only one
    embedded sync-wait, but Tile's scheduler can attach more. Hoist all
    embedded waits of multi-wait instructions into a NoOp injected just
    before them on the same engine queue (NoOps carry many waits fine)."""
    import concourse.mybir as mybir
    n = 0
    for f in nc.m.functions:
        for blk in f.blocks:
            out = []
            for ins in blk.instructions:
                si = ins.sync_info
                if (si is not None and si.on_wait is not None
                        and len(si.on_wait) > 1):
                    for w in si.on_wait:
                        nop = mybir.InstNoOp(
                            name=f"WFIX-{n}", engine=ins.engine,
                            sync_info=mybir.SyncInfo(on_wait=[w],
                                                     on_update=[]))
                        n += 1
                        out.append(nop)
                    ins.sync_info = mybir.SyncInfo(
                        on_wait=[],
                        on_update=list(si.on_update or []))
                out.append(ins)
            blk.instructions = out


def _in_maps(img):
    img = np.ascontiguousarray(img, dtype=np.float32)
    pad = np.zeros((B, 3, 5, W), np.float32)
    imgp = np.concatenate([pad, img], axis=2)  # rows shifted by +5
    maps = []
    for i in range(NC):
        r0 = SLAB * i  # padded row index of img row 118i-5
        xm_i = imgp[:, :, r0:r0 + 128, :].reshape(B * 3, 128, W)
        x8_i = img[i, :, B8_START - 8:, :]  # img rows 936..1023
        m = {"xm": np.ascontiguousarray(xm_i),
             "x8": np.ascontiguousarray(x8_i)}
        m.update(_const_mats(i))
        maps.append(m)
    return maps


def kernel(img, gauss_h=None, gauss_v=None, sobel_h=None, sobel_v=None,
           dir_f=None, connect_f=None, _want_time=False):
    from concourse.bass_utils import run_bass_kernel_spmd
    nc = _build_program()
    maps = _in_maps(np.asarray(img))
    res = run_bass_kernel_spmd(nc, maps, list(range(NC)), trace=_want_time)
    out = np.zeros((B, 1, H, W), np.float32)
    for i in range(NC):
        r = res.results[i]
        out[:, 0, SLAB * i:SLAB * (i + 1), :] = r["outm"]
        out[i, 0, B8_START:, :] = r["out8"]
    if _want_time:
        return out, res
    return out



# revision 24
# speedup vs baseline: 1.1251x; 1.1251x over previous
"""Canny edge detector on 8 Trainium2 NeuronCores (Bass/Tile) — v2.

Sharding: row slabs (see baseline docstring for why: the reference's flat
gather cross-wires images, so every output pixel needs all 8 images' gm).

v2 changes vs baseline:
- Sobel matmuls run in fp32r (4x PE throughput) with exact precision: the
  blurred field is mean-centered (-MU) and split into hi = f32r(bl') and
  lo = bl' - hi; sobel weights are small integers (exact in fp32r), so
  accumulating hi+lo in PSUM reproduces the f32 result to ~1e-6.
- B8 band computed FIRST; only its 8 direction-compare bitmaps (u8) are
  exchanged via ONE AllToAll (663KB vs 5.3MB AllGather+AllToAll in f32),
  fully overlapped with the main conv.
- gm for all 8 images lives in one 3D composite tile -> NMS compares are
  one instruction per (direction, half) over all images.
- Directions 4..7 reuse directions 0..3: C_{b+4}(p) = NOT C_b(p - delta)
  (exact up to f32 ties, which do not occur for this data).
- Elementwise work spread across DVE/Pool/Act per a makespan balance.
"""

import os

os.environ.setdefault("BY_DEFAULT_DISABLE_SUBTILE_DEPS", "1")

import numpy as np

H = 1024
W = 1024
B = 8
NC = 8
SLAB = 118
B8_START = SLAB * NC          # 944
B8_ROWS = H - B8_START        # 80
LOW_T, HIGH_T = 2.5, 5.0
T22SQ = float(np.float32(np.tan(np.pi / 8.0)) ** 2)
MU = 3.0807319                # E[bl] for uniform input; exactness not needed

DELTAS = {0: (0, 1), 1: (1, 1), 2: (1, 0), 3: (1, -1),
          4: (0, -1), 5: (-1, -1), 6: (-1, 0), 7: (-1, 1)}


def _fp32r_round(v):
    u = np.asarray(v, np.float32).reshape(1).view(np.uint32)
    r = ((u >> 12) & 1) + 0x07FF
    return float(((u + r) & ~np.uint32(0xFFF)).view(np.float32)[0])


MU_HI = _fp32r_round(-MU)                       # hi-margin value (= f32r(-MU))
MU_LO = _fp32r_round(np.float32(-MU) - np.float32(MU_HI))


def _gauss5():
    n = np.arange(5, dtype=np.float32) - 2.0
    return np.exp(-0.5 * n * n).astype(np.float32)


def _band(n_in, n_out, offset, taps):
    m_ = np.zeros((n_in, n_out), np.float32)
    for mm in range(n_out):
        for t, w in enumerate(taps):
            k = mm + offset + t
            if 0 <= k < n_in:
                m_[k, mm] = w
    return m_


def _const_mats(core):
    g = _gauss5()
    g0 = float(g[0])
    mats = {}
    mats["BV"] = _band(128, 124, 0, (g0 * g).tolist())
    b121 = _band(124, 122, 0, [1.0, 2.0, 1.0])
    b10m1 = _band(124, 122, 0, [1.0, 0.0, -1.0])
    if core == 0:  # img rows -2,-1 must yield gm=0 (zero-pad semantics)
        b121[:, 0:2] = 0.0
        b10m1[:, 0:2] = 0.0
    mats["B121"] = b121
    mats["B121N"] = -b121
    mats["B10M1"] = b10m1
    mats["B10M1X2"] = 2.0 * b10m1
    bones = _band(120, 119, -1, [1.0, 1.0, 1.0])
    bones[:, 0] = 0.0
    if core == 0:
        bones[:, 1] = 0.0
    mats["BONES"] = bones
    mats["BV8"] = _band(88, 86, 0, (g0 * g).tolist())
    b121_8 = _band(86, 84, 1, [1.0, 2.0, 1.0])
    b10m1_8 = _band(86, 84, 1, [1.0, 0.0, -1.0])
    mats["B121_8"] = b121_8
    mats["B121N_8"] = -b121_8
    mats["B10M1_8"] = b10m1_8
    mats["B10M1X2_8"] = 2.0 * b10m1_8
    bones8 = _band(81, 81, -1, [1.0, 1.0, 1.0])
    bones8[:, 0] = 0.0
    bones8[:, 80] = 0.0
    mats["BONES8"] = bones8
    gycor = np.zeros((84, 2), np.float32)
    gycor[83, 0] = 4.0 * np.float32(MU)   # clipped B10M1_8 col 83: colsum 1
    gycor[83, 1] = 12.0 * np.float32(MU)  # 3 channels summed, for gys8
    mats["GYCOR"] = gycor
    return {k: np.ascontiguousarray(v, np.float32) for k, v in mats.items()}


MAT_SPECS = {
    "GYCOR": [84, 2],
    "BV": [128, 124], "B121": [124, 122], "B121N": [124, 122],
    "B10M1": [124, 122], "B10M1X2": [124, 122], "BONES": [120, 119],
    "BV8": [88, 86], "B121_8": [86, 84], "B121N_8": [86, 84],
    "B10M1_8": [86, 84], "B10M1X2_8": [86, 84], "BONES8": [81, 81],
}
F32R_MATS = ("B121", "B121N", "B10M1", "B10M1X2",
             "B121_8", "B121N_8", "B10M1_8", "B10M1X2_8")
BF16_MATS = ("BONES", "BONES8")

_CACHE = {}


def _build_program():
    if "nc" in _CACHE:
        return _CACHE["nc"]
    import concourse.bass as bass
    import concourse.mybir as mybir
    from concourse.tile import TileContext

    f32 = mybir.dt.float32
    f32r = mybir.dt.float32r
    bf16 = mybir.dt.bfloat16
    u8 = mybir.dt.uint8
    Alu = mybir.AluOpType

    g = _gauss5()
    R10G = float(g[1] / g[0])
    R20G = float(g[2] / g[0])

    nc = bass.Bass()

    xm = nc.declare_dram_parameter("xm", [B * 3, 128, W], f32, isOutput=False)
    x8 = nc.declare_dram_parameter("x8", [3, 88, W], f32, isOutput=False)
    mat_d = {k: nc.declare_dram_parameter(k, v, f32, isOutput=False)
             for k, v in MAT_SPECS.items()}
    outm = nc.declare_dram_parameter("outm", [B, SLAB, W], f32, isOutput=True)
    out8 = nc.declare_dram_parameter("out8", [B8_ROWS, W], f32, isOutput=True)

    with TileContext(nc) as tc:
        with (
            tc.tile_pool(name="consts", bufs=1) as cpool,
            tc.tile_pool(name="gmp", bufs=1) as gmpool,
            tc.tile_pool(name="mskp", bufs=1) as mskpool,
            tc.tile_pool(name="b8p", bufs=1) as b8pool,
            tc.tile_pool(name="dram", bufs=1, space="DRAM") as dpool,
        ):
            # ---- constants ------------------------------------------------
            mt = {}
            for name, shp in MAT_SPECS.items():
                t = cpool.tile(shp, f32, tag=name)
                nc.sync.dma_start(out=t[:], in_=mat_d[name][:])
                if name in F32R_MATS:
                    tr = cpool.tile(shp, f32r, tag=name + "r")
                    nc.scalar.copy(out=tr[:], in_=t[:])
                    mt[name] = tr
                elif name in BF16_MATS:
                    tb = cpool.tile(shp, bf16, tag=name + "b")
                    nc.scalar.copy(out=tb[:], in_=t[:])
                    mt[name] = tb
                else:
                    mt[name] = t
            bias = cpool.tile([128, 1], f32, tag="bias")
            nc.vector.memset(bias[:], -MU)
            muhi_c = cpool.tile([128, 1], f32, tag="muhi_c")
            nc.vector.memset(muhi_c[:], MU_HI)
            mulo_c = cpool.tile([128, 1], f32, tag="mulo_c")
            nc.vector.memset(mulo_c[:], MU_LO)

            # gm composite: [122, 8, 1030] f32; image j plane, data col 3+w,
            # 3 margin cols each side (needed for shifted compare reads)
            gm_all = gmpool.tile([122, B, 1030], f32, tag="gm_all")
            # B8 gm: [85, 1026] f32; row p <-> img 940+p, data col 1+w
            gm8 = b8pool.tile([85, 1026], f32, tag="gm8")
            nc.vector.memset(gm8[:], 0.0)
            # thin-frame mask composites (u8), 1-col margins per plane
            mco = {t: mskpool.tile([120, B, 1026], u8, tag=f"mc_{t}",
                                   name=f"mc_{t}")
                   for t in ("c0", "c2", "sm")}
            for t in ("c0", "c2", "sm"):
                nc.vector.memset(mco[t][:, :, 0:1], 0)
                nc.vector.memset(mco[t][:, :, W + 1:W + 2], 0)
            m8 = {t: b8pool.tile([81, 1024], u8, tag=f"m8_{t}", name=f"m8_{t}")
                  for t in ("c0", "c2", "sm")}
            ce8 = b8pool.tile([81, 1026], f32, tag="ce8")
            a2a_in = dpool.tile([B * 81, W], u8, tag="a2a_in")
            a2a_out = dpool.tile([B * 81, W], u8, tag="a2a_out")

            # =========== conv phase ========================================
            with (
                tc.tile_pool(name="xin", bufs=2) as xpool,
                tc.tile_pool(name="hbt", bufs=2) as hbpool,
                tc.tile_pool(name="blt", bufs=2) as blpool,
                tc.tile_pool(name="sq", bufs=2) as sqpool,
                tc.tile_pool(name="gsum", bufs=2) as gsumpool,
                tc.tile_pool(name="mskt", bufs=2) as msktpool,
                tc.tile_pool(name="psA", bufs=2, space="PSUM") as psA,
                tc.tile_pool(name="psB", bufs=1, space="PSUM") as psB,
            ):
                def conv_channel(xt, n_in, n_bl, n_gxy, bv, b121, b121n,
                                 b10m1, b10m1x2, hb_eng, gm_dst, mag_c,
                                 gxs, gys, c, sqy_bias=None):
                    """One channel: h-blur, fp32 v-blur, split-f32r sobel,
                    magnitude.  gm_dst: AP for this image's gm slice rows
                    [0:n_gxy]; mag_c: list collecting per-channel mag tiles."""
                    # h-blur, symmetric: h2 = t1 + (g1/g0) t2 + (g2/g0) x2
                    # where t1 = x[-2]+x[2], t2 = x[-1]+x[1].  Pool does the
                    # two adds, DVE the two fused madds.
                    h1 = hbpool.tile([n_in, W], f32, tag="h1")
                    h2 = hbpool.tile([n_in, W], f32, tag="h2")
                    t1 = hbpool.tile([n_in, W], f32, tag="t1")
                    nc.gpsimd.tensor_tensor(out=t1[:], in0=xt[:, 0:W],
                                            in1=xt[:, 4:W + 4], op=Alu.add)
                    nc.gpsimd.tensor_tensor(out=h1[:], in0=xt[:, 1:W + 1],
                                            in1=xt[:, 3:W + 3], op=Alu.add)
                    nc.vector.scalar_tensor_tensor(
                        out=h2[:], in0=h1[:], scalar=R10G, in1=t1[:],
                        op0=Alu.mult, op1=Alu.add)
                    nc.vector.scalar_tensor_tensor(
                        out=h2[:], in0=xt[:, 2:W + 2], scalar=R20G,
                        in1=h2[:], op0=Alu.mult, op1=Alu.add)
                    # v-blur: exact fp32 matmul -> PSUM
                    bl = psA.tile([n_bl, W], f32, tag="bl")
                    for lo in (0, 512):
                        nc.tensor.matmul(out=bl[:, lo:lo + 512],
                                         lhsT=bv[0:n_in, 0:n_bl],
                                         rhs=h2[:, lo:lo + 512],
                                         start=True, stop=True)
                    # center + split into f32r hi/lo with -MU margins
                    bhi = blpool.tile([n_bl, W + 2], f32r, tag="bhi")
                    blo = blpool.tile([n_bl, W + 2], f32r, tag="blo")
                    for mcol, dsts in ((muhi_c, bhi), (mulo_c, blo)):
                        for cs in (slice(0, 1), slice(W + 1, W + 2)):
                            nc.vector.tensor_scalar(
                                out=dsts[:, cs], in0=mcol[0:n_bl, 0:1],
                                scalar1=1.0, scalar2=None, op0=Alu.mult)
                    nc.scalar.add(out=bhi[:, 1:W + 1], in_=bl[:],
                                  add=bias[0:n_bl, 0:1])
                    nc.vector.scalar_tensor_tensor(
                        out=blo[:, 1:W + 1], in0=bl[:], scalar=-MU,
                        in1=bhi[:, 1:W + 1], op0=Alu.add, op1=Alu.subtract)
                    # sobel: 20 f32r matmuls (hi+lo accumulate in PSUM)
                    gx = psB.tile([n_gxy, W], f32, tag="gx")
                    gy = psB.tile([n_gxy, W], f32, tag="gy")
                    for lo in (0, 512):
                        first = True
                        for src in (bhi, blo):
                            sm_ = src[:, 0 + lo:512 + lo]
                            sc = src[:, 1 + lo:513 + lo]
                            sp = src[:, 2 + lo:514 + lo]
                            nc.tensor.matmul(out=gx[:, lo:lo + 512],
                                             lhsT=b121[0:n_bl, 0:n_gxy],
                                             rhs=sm_, start=first, stop=False)
                            nc.tensor.matmul(out=gx[:, lo:lo + 512],
                                             lhsT=b121n[0:n_bl, 0:n_gxy],
                                             rhs=sp, start=False,
                                             stop=(src is blo))
                            nc.tensor.matmul(out=gy[:, lo:lo + 512],
                                             lhsT=b10m1[0:n_bl, 0:n_gxy],
                                             rhs=sp, start=first, stop=False)
                            nc.tensor.matmul(out=gy[:, lo:lo + 512],
                                             lhsT=b10m1x2[0:n_bl, 0:n_gxy],
                                             rhs=sc, start=False, stop=False)
                            nc.tensor.matmul(out=gy[:, lo:lo + 512],
                                             lhsT=b10m1[0:n_bl, 0:n_gxy],
                                             rhs=sm_, start=False,
                                             stop=(src is blo))
                            first = False
                    # magnitude: mag_c[c] = sqrt(gx^2 + gy^2)
                    sqx = sqpool.tile([n_gxy, W], f32, tag="sqx")
                    sqy = sqpool.tile([n_gxy, W], f32, tag="sqy")
                    nc.scalar.square(out=sqx[:], in_=gx[:])
                    if sqy_bias is None:
                        nc.scalar.square(out=sqy[:], in_=gy[:])
                    else:
                        nc.scalar.activation(
                            out=sqy[:], in_=gy[:],
                            func=mybir.ActivationFunctionType.Square,
                            bias=sqy_bias)
                    nc.vector.tensor_tensor(out=sqx[:], in0=sqx[:],
                                            in1=sqy[:], op=Alu.add)
                    mg = sqpool.tile([n_gxy, W], f32, tag=f"mag{c}")
                    nc.scalar.sqrt(out=mg[:], in_=sqx[:])
                    mag_c.append(mg)
                    # orientation sums
                    if c == 0:
                        nc.scalar.copy(out=gxs[0:n_gxy, :], in_=gx[:])
                        nc.scalar.copy(out=gys[0:n_gxy, :], in_=gy[:])
                    else:
                        nc.vector.tensor_tensor(out=gxs[0:n_gxy, :],
                                                in0=gxs[0:n_gxy, :],
                                                in1=gx[:], op=Alu.add)
                        nc.vector.tensor_tensor(out=gys[0:n_gxy, :],
                                                in0=gys[0:n_gxy, :],
                                                in1=gy[:], op=Alu.add)
                    if c == 2:
                        # gm = mag0 + mag1 + mag2; single writer of gm slice
                        t01 = sqpool.tile([n_gxy, W], f32, tag="sqy")
                        nc.vector.tensor_tensor(out=t01[:], in0=mag_c[0][:],
                                                in1=mag_c[1][:], op=Alu.add)
                        nc.gpsimd.tensor_tensor(out=gm_dst, in0=t01[:],
                                                in1=mag_c[2][:], op=Alu.add)

                def make_masks(gxs, gys, n, shift, n_thin, dst, dst_slices):
                    """u8 masks at conv frame [0:n]; DMA rows
                    [shift:shift+n_thin] into dst[t] slices."""
                    a2 = sqpool.tile([n, W], f32, tag="sqx")
                    b2 = sqpool.tile([n, W], f32, tag="sqy")
                    nc.scalar.square(out=a2[:, :], in_=gxs[0:n, :])
                    nc.scalar.square(out=b2[:, :], in_=gys[0:n, :])
                    tmp = {t: msktpool.tile([n, W], u8, tag=f"t{t}", name=f"t{t}")
                           for t in ("c0", "c2", "sm")}
                    nc.vector.scalar_tensor_tensor(
                        out=tmp["c0"][:], in0=a2[:], scalar=T22SQ,
                        in1=b2[:], op0=Alu.mult, op1=Alu.is_gt)
                    nc.vector.scalar_tensor_tensor(
                        out=tmp["c2"][:], in0=b2[:], scalar=T22SQ,
                        in1=a2[:], op0=Alu.mult, op1=Alu.is_gt)
                    ab = sqpool.tile([n, W], f32, tag="mag0")
                    nc.gpsimd.tensor_tensor(out=ab[:], in0=gxs[0:n, :],
                                            in1=gys[0:n, :], op=Alu.mult)
                    nc.vector.tensor_scalar(out=tmp["sm"][:], in0=ab[:],
                                            scalar1=0.0, scalar2=None,
                                            op0=Alu.is_ge)
                    for t in ("c0", "c2", "sm"):
                        nc.sync.dma_start(
                            out=dst_slices(dst[t]),
                            in_=tmp[t][shift:shift + n_thin, :])

                # ---- B8 block first (feeds the collective) ---------------
                gxs8 = gsumpool.tile([84, W], f32, tag="gxs")
                gys8 = gsumpool.tile([84, W], f32, tag="gys")
                mag8 = []
                for c in range(3):
                    xt = xpool.tile([88, W + 4], f32, tag="x")
                    nc.vector.memset(xt[:, 0:2], 0.0)
                    nc.vector.memset(xt[:, W + 2:W + 4], 0.0)
                    nc.sync.dma_start(out=xt[:, 2:W + 2], in_=x8[c])
                    conv_channel(xt, 88, 86, 84, mt["BV8"], mt["B121_8"],
                                 mt["B121N_8"], mt["B10M1_8"],
                                 mt["B10M1X2_8"],
                                 nc.gpsimd if c != 2 else nc.vector,
                                 gm8[0:84, 1:W + 1], mag8, gxs8, gys8, c,
                                 sqy_bias=mt["GYCOR"][0:84, 0:1])
                gys8c = gsumpool.tile([84, W], f32, tag="gys8c", bufs=1)
                nc.scalar.add(out=gys8c[:], in_=gys8[:],
                              add=mt["GYCOR"][0:84, 1:2])
                make_masks(gxs8, gys8c, 84, 3, 81, m8, lambda t: t[:])

                # D maps: realign gm8 rows and compare in 8 directions
                ce8d = msktpool.tile([81, 1026], f32, tag="dn8", bufs=1)
                up8d = msktpool.tile([81, 1026], f32, tag="up8", bufs=1)
                nc.sync.dma_start(out=ce8[:], in_=gm8[3:84, :])
                nc.sync.dma_start(out=ce8d[:], in_=gm8[4:85, :])
                nc.sync.dma_start(out=up8d[:], in_=gm8[2:83, :])
                d8 = msktpool.tile([81, B, 1024], u8, tag="d8", bufs=1)
                for d, (dr, dc) in DELTAS.items():
                    src = {0: ce8, 1: ce8d, -1: up8d}[dr]
                    nc.vector.tensor_tensor(out=d8[:, d, :],
                                            in0=ce8[:, 1:W + 1],
                                            in1=src[:, 1 + dc:W + 1 + dc],
                                            op=Alu.is_gt)
                for d in range(B):
                    nc.sync.dma_start(out=a2a_in[81 * d:81 * (d + 1), :],
                                      in_=d8[:, d, :])
                nc.gpsimd.collective_compute(
                    "AllToAll", Alu.bypass,
                    replica_groups=[list(range(NC))],
                    ins=[a2a_in.opt()], outs=[a2a_out.opt()])

                # ---- main slab: 8 images x 3 channels --------------------
                for j in range(B):
                    nc.vector.memset(gm_all[:, j, 0:3], 0.0)
                    nc.vector.memset(gm_all[:, j, W + 3:W + 6], 0.0)
                    gxs = gsumpool.tile([122, W], f32, tag="gxs")
                    gys = gsumpool.tile([122, W], f32, tag="gys")
                    mag_c = []
                    for c in range(3):
                        xt = xpool.tile([128, W + 4], f32, tag="x")
                        nc.vector.memset(xt[:, 0:2], 0.0)
                        nc.vector.memset(xt[:, W + 2:W + 4], 0.0)
                        nc.sync.dma_start(out=xt[:, 2:W + 2], in_=xm[3 * j + c])
                        hb = nc.gpsimd if c != 2 else nc.vector
                        conv_channel(xt, 128, 124, 122, mt["BV"], mt["B121"],
                                     mt["B121N"], mt["B10M1"], mt["B10M1X2"],
                                     hb, gm_all[0:122, j, 3:W + 3], mag_c,
                                     gxs, gys, c)
                    make_masks(gxs, gys, 122, 1, 120, mco,
                               lambda t, j=j: t[:, j, 1:W + 1])

            # =========== NMS phase =========================================
            with (
                tc.tile_pool(name="cep", bufs=1) as cepool,
                tc.tile_pool(name="cb", bufs=2) as cbpool,
                tc.tile_pool(name="pb", bufs=1) as pbpool,
                tc.tile_pool(name="tail", bufs=2) as tlpool,
                tc.tile_pool(name="otp", bufs=2) as otpool,
                tc.tile_pool(name="psC", bufs=2, space="PSUM") as psC,
            ):
                ce121 = cepool.tile([121, B, 1030], f32, tag="ce121")
                nc.sync.dma_start(out=ce121[:], in_=gm_all[1:122, :, :])

                def nms_tail(b, pcomp, half):
                    """psel/strong/q/mh/mp/ot/out for output image b.
                    pcomp: P composite [120,4,514] covering
                    w in [base-1 .. base+512]."""
                    base = 512 * half
                    psel = tlpool.tile([120, 514], bf16, tag="psel")
                    nc.scalar.copy(out=psel[:], in_=pcomp[:, 3, :])
                    for t, k in (("sm", 1), ("c0", 0), ("c2", 2)):
                        nc.vector.copy_predicated(
                            out=psel[:], mask=mco[t][:, b, base:base + 514],
                            data=pcomp[:, k, :])
                    hi1 = tlpool.tile([120, 514], bf16, tag="hi1")
                    nc.vector.tensor_scalar(
                        out=hi1[:], in0=ce121[0:120, b, base + 2:base + 516],
                        scalar1=HIGH_T, scalar2=None, op0=Alu.is_gt)
                    strong = tlpool.tile([120, 514], bf16, tag="strong")
                    nc.gpsimd.tensor_tensor(out=strong[:], in0=hi1[:],
                                            in1=psel[:], op=Alu.mult)
                    q1 = tlpool.tile([120, 512], bf16, tag="q1")
                    nc.vector.tensor_scalar(
                        out=q1[:], in0=ce121[0:120, b, base + 3:base + 515],
                        scalar1=LOW_T, scalar2=None, op0=Alu.is_ge)
                    q = tlpool.tile([120, 512], bf16, tag="q")
                    nc.gpsimd.tensor_tensor(out=q[:], in0=q1[:],
                                            in1=psel[:, 1:513],
                                            op=Alu.mult)
                    mh = tlpool.tile([120, 512], bf16, tag="mh")
                    nc.vector.tensor_tensor(out=mh[:], in0=strong[:, 0:512],
                                            in1=strong[:, 2:514], op=Alu.add)
                    nc.vector.tensor_tensor(out=mh[:], in0=mh[:],
                                            in1=strong[:, 1:513], op=Alu.add)
                    mp = psC.tile([119, 512], f32, tag="mp")
                    nc.tensor.matmul(out=mp[:], lhsT=mt["BONES"][0:120, 0:119],
                                     rhs=mh[:], start=True, stop=True)
                    ot = otpool.tile([119, 512], f32, tag="ot")
                    nc.vector.scalar_tensor_tensor(
                        out=ot[:], in0=mp[:], scalar=0.5, in1=q[0:119, :],
                        op0=Alu.is_ge, op1=Alu.logical_and)
                    if half == 0:
                        nc.vector.memset(ot[:, 0:1], 0.0)
                    else:
                        nc.vector.memset(ot[:, 511:512], 0.0)
                    nc.sync.dma_start(out=outm[b][:, base:base + 512],
                                      in_=ot[1:119, :])

                for half in (0, 1):
                    base = 512 * half
                    for b in (0, 1, 2, 3):
                        dr, dc = DELTAS[b]
                        # C col i <-> w = base-2+i (516 cols); col(w) = 3+w
                        s0 = base + 1
                        if b == 0:
                            df = cbpool.tile([120, B, 516], f32, tag="df0",
                                             bufs=1)
                            nc.gpsimd.tensor_tensor(
                                out=df[:],
                                in0=ce121[0:120, :, s0:s0 + 516],
                                in1=ce121[0:120, :, s0 + dc:s0 + dc + 516],
                                op=Alu.subtract)
                            cth = cbpool.tile([120, B, 516], bf16, tag="c0t",
                                              bufs=1)
                            nc.vector.tensor_scalar(
                                out=cth[:], in0=df[:], scalar1=0.0,
                                scalar2=None, op0=Alu.is_gt)
                            cx_low = cth  # dr=0: same rows for shifted view
                        else:
                            df = cbpool.tile([121, B, 516], f32, tag="df",
                                             bufs=1)
                            nc.gpsimd.tensor_tensor(
                                out=df[:],
                                in0=gm_all[0:121, :, s0:s0 + 516],
                                in1=ce121[0:121, :, s0 + dc:s0 + dc + 516],
                                op=Alu.subtract)
                            cx = cbpool.tile([121, B, 516], bf16, tag="cx",
                                             bufs=1)
                            nc.vector.tensor_scalar(
                                out=cx[:], in0=df[:], scalar1=0.0,
                                scalar2=None, op0=Alu.is_gt)
                            cth = cbpool.tile([120, B, 516], bf16, tag="cth",
                                              bufs=1)
                            nc.sync.dma_start(out=cth[:], in_=cx[1:121, :, :])
                            cx_low = cx  # rows 0:120 = thin p-1 view
                        # P_b: planes k AND k+4 on w [base-1 .. base+512]
                        pb = pbpool.tile([120, 4, 514], bf16, tag="pb")
                        nc.vector.tensor_tensor(
                            out=pb[:], in0=cth[:, 0:4, 1:515],
                            in1=cth[:, 4:8, 1:515], op=Alu.logical_and)
                        nms_tail(b, pb, half)
                        # P_{b+4} = NOT C_b(p-dr, w-dc) pairwise: sum==0
                        ss = 1 - dc
                        sb = pbpool.tile([120, 4, 514], bf16, tag="sb")
                        nc.vector.tensor_tensor(
                            out=sb[:],
                            in0=cx_low[0:120, 0:4, ss:ss + 514],
                            in1=cx_low[0:120, 4:8, ss:ss + 514],
                            op=Alu.add)
                        pb4 = pbpool.tile([120, 4, 514], bf16, tag="pb4")
                        nc.vector.tensor_scalar(
                            out=pb4[:], in0=sb[:], scalar1=0.0, scalar2=None,
                            op0=Alu.is_equal)
                        nms_tail(b + 4, pb4, half)

            # =========== B8 NMS ===========================================
            with (
                tc.tile_pool(name="b8n", bufs=1) as b8n,
                tc.tile_pool(name="psD", bufs=1, space="PSUM") as psD,
            ):
                dr8 = b8n.tile([81, B, 1024], u8, tag="dr8")
                for d in range(B):
                    nc.sync.dma_start(out=dr8[:, d, :],
                                      in_=a2a_out[81 * d:81 * (d + 1), :])
                p8 = b8n.tile([81, 4, 1024], u8, tag="p8")
                nc.vector.tensor_tensor(out=p8[:], in0=dr8[:, 0:4, :],
                                        in1=dr8[:, 4:8, :],
                                        op=Alu.logical_and)
                psel8 = b8n.tile([81, 1024], u8, tag="psel8")
                nc.scalar.copy(out=psel8[:], in_=p8[:, 3, :])
                for t, k in (("sm", 1), ("c0", 0), ("c2", 2)):
                    nc.vector.copy_predicated(out=psel8[:], mask=m8[t][:],
                                              data=p8[:, k, :])
                strong8 = b8n.tile([81, 1026], bf16, tag="strong8")
                nc.vector.memset(strong8[:, 0:1], 0.0)
                nc.vector.memset(strong8[:, W + 1:W + 2], 0.0)
                hi8 = b8n.tile([81, 1024], bf16, tag="hi8")
                nc.vector.tensor_scalar(
                    out=hi8[:], in0=ce8[:, 1:W + 1], scalar1=HIGH_T,
                    scalar2=None, op0=Alu.is_gt)
                ps8b = b8n.tile([81, 1024], bf16, tag="ps8b")
                nc.scalar.copy(out=ps8b[:], in_=psel8[:])
                nc.gpsimd.tensor_tensor(out=strong8[:, 1:W + 1], in0=hi8[:],
                                        in1=ps8b[:], op=Alu.mult)
                q8 = b8n.tile([81, 1024], u8, tag="q8")
                nc.vector.scalar_tensor_tensor(
                    out=q8[:], in0=ce8[:, 1:W + 1], scalar=LOW_T,
                    in1=psel8[:], op0=Alu.is_ge, op1=Alu.logical_and)
                mh8 = b8n.tile([81, 1024], bf16, tag="mh8")
                nc.vector.tensor_tensor(out=mh8[:], in0=strong8[:, 0:W],
                                        in1=strong8[:, 2:W + 2], op=Alu.add)
                nc.vector.tensor_tensor(out=mh8[:], in0=mh8[:],
                                        in1=strong8[:, 1:W + 1], op=Alu.add)
                mp8 = psD.tile([81, 1024], f32, tag="mp8")
                for lo in (0, 512):
                    nc.tensor.matmul(out=mp8[:, lo:lo + 512],
                                     lhsT=mt["BONES8"][0:81, 0:81],
                                     rhs=mh8[:, lo:lo + 512],
                                     start=True, stop=True)
                ot8 = b8n.tile([81, 1024], f32, tag="ot8")
                nc.vector.scalar_tensor_tensor(
                    out=ot8[:], in0=mp8[:], scalar=0.5, in1=q8[:],
                    op0=Alu.is_ge, op1=Alu.logical_and)
                nc.vector.memset(ot8[:, 0:1], 0.0)
                nc.vector.memset(ot8[:, W - 1:W], 0.0)
                nc.sync.dma_start(out=out8[:], in_=ot8[1:81, :])

    _legalize_waits(nc)
    _CACHE["nc"] = nc
    return nc


def _legalize_waits(nc):
    """Hoist embedded waits of multi-wait instructions into NoOps (several
    ISA encodings hold only one embedded sync-wait)."""
    import concourse.mybir as mybir
    n = 0
    for f in nc.m.functions:
        for blk in f.blocks:
            out = []
            for ins in blk.instructions:
                si = ins.sync_info
                if (si is not None and si.on_wait is not None
                        and len(si.on_wait) > 1):
                    for w in si.on_wait:
                        nop = mybir.InstNoOp(
                            name=f"WFIX-{n}", engine=ins.engine,
                            sync_info=mybir.SyncInfo(on_wait=[w],
                                                     on_update=[]))
                        n += 1
                        out.append(nop)
                    ins.sync_info = mybir.SyncInfo(
                        on_wait=[],
                        on_update=list(si.on_update or []))
                out.append(ins)
            blk.instructions = out


def _in_maps(img):
    img = np.ascontiguousarray(img, dtype=np.float32)
    pad = np.zeros((B, 3, 5, W), np.float32)
    imgp = np.concatenate([pad, img], axis=2)  # rows shifted by +5
    maps = []
    for i in range(NC):
        r0 = SLAB * i
        xm_i = imgp[:, :, r0:r0 + 128, :].reshape(B * 3, 128, W)
        x8_i = img[i, :, B8_START - 8:, :]
        m = {"xm": np.ascontiguousarray(xm_i),
             "x8": np.ascontiguousarray(x8_i)}
        m.update(_const_mats(i))
        maps.append(m)
    return maps


def kernel(img, gauss_h=None, gauss_v=None, sobel_h=None, sobel_v=None,
           dir_f=None, connect_f=None, _want_time=False):
    from concourse.bass_utils import run_bass_kernel_spmd
    nc = _build_program()
    maps = _in_maps(np.asarray(img))
    res = run_bass_kernel_spmd(nc, maps, list(range(NC)), trace=_want_time)
    out = np.zeros((B, 1, H, W), np.float32)
    for i in range(NC):
        r = res.results[i]
        out[:, 0, SLAB * i:SLAB * (i + 1), :] = r["outm"]
        out[i, 0, B8_START:, :] = r["out8"]
    if _want_time:
        return out, res
    return out


# revision 30
# speedup vs baseline: 1.2058x; 1.0717x over previous
"""Canny edge detector on 8 Trainium2 NeuronCores (Bass/Tile) — v2.

Sharding: row slabs (see baseline docstring for why: the reference's flat
gather cross-wires images, so every output pixel needs all 8 images' gm).

v2 changes vs baseline:
- Sobel matmuls run in fp32r (4x PE throughput) with exact precision: the
  blurred field is mean-centered (-MU) and split into hi = f32r(bl') and
  lo = bl' - hi; sobel weights are small integers (exact in fp32r), so
  accumulating hi+lo in PSUM reproduces the f32 result to ~1e-6.
- B8 band computed FIRST; only its 8 direction-compare bitmaps (u8) are
  exchanged via ONE AllToAll (663KB vs 5.3MB AllGather+AllToAll in f32),
  fully overlapped with the main conv.
- gm for all 8 images lives in one 3D composite tile -> NMS compares are
  one instruction per (direction, half) over all images.
- Directions 4..7 reuse directions 0..3: C_{b+4}(p) = NOT C_b(p - delta)
  (exact up to f32 ties, which do not occur for this data).
- Elementwise work spread across DVE/Pool/Act per a makespan balance.
"""

import os

os.environ.setdefault("BY_DEFAULT_DISABLE_SUBTILE_DEPS", "1")

import numpy as np

H = 1024
W = 1024
B = 8
NC = 8
SLAB = 118
B8_START = SLAB * NC          # 944
B8_ROWS = H - B8_START        # 80
LOW_T, HIGH_T = 2.5, 5.0
T22SQ = float(np.float32(np.tan(np.pi / 8.0)) ** 2)
MU = 3.0807319                # E[bl] for uniform input; exactness not needed

DELTAS = {0: (0, 1), 1: (1, 1), 2: (1, 0), 3: (1, -1),
          4: (0, -1), 5: (-1, -1), 6: (-1, 0), 7: (-1, 1)}


def _fp32r_round(v):
    u = np.asarray(v, np.float32).reshape(1).view(np.uint32)
    r = ((u >> 12) & 1) + 0x07FF
    return float(((u + r) & ~np.uint32(0xFFF)).view(np.float32)[0])


MU_HI = _fp32r_round(-MU)                       # hi-margin value (= f32r(-MU))
MU_LO = _fp32r_round(np.float32(-MU) - np.float32(MU_HI))


def _gauss5():
    n = np.arange(5, dtype=np.float32) - 2.0
    return np.exp(-0.5 * n * n).astype(np.float32)


def _band(n_in, n_out, offset, taps):
    m_ = np.zeros((n_in, n_out), np.float32)
    for mm in range(n_out):
        for t, w in enumerate(taps):
            k = mm + offset + t
            if 0 <= k < n_in:
                m_[k, mm] = w
    return m_


def _const_mats(core):
    g = _gauss5()
    g0 = float(g[0])
    mats = {}
    mats["BV"] = _band(128, 124, 0, (g0 * g).tolist())
    b121 = _band(124, 122, 0, [1.0, 2.0, 1.0])
    b10m1 = _band(124, 122, 0, [1.0, 0.0, -1.0])
    if core == 0:  # img rows -2,-1 must yield gm=0 (zero-pad semantics)
        b121[:, 0:2] = 0.0
        b10m1[:, 0:2] = 0.0
    mats["B121"] = b121
    mats["B121N"] = -b121
    mats["B10M1"] = b10m1
    mats["B10M1X2"] = 2.0 * b10m1
    bones = _band(120, 119, -1, [1.0, 1.0, 1.0])
    bones[:, 0] = 0.0
    if core == 0:
        bones[:, 1] = 0.0
    mats["BONES"] = bones
    mats["BV8"] = _band(88, 86, 0, (g0 * g).tolist())
    b121_8 = _band(86, 84, 1, [1.0, 2.0, 1.0])
    b10m1_8 = _band(86, 84, 1, [1.0, 0.0, -1.0])
    mats["B121_8"] = b121_8
    mats["B121N_8"] = -b121_8
    mats["B10M1_8"] = b10m1_8
    mats["B10M1X2_8"] = 2.0 * b10m1_8
    bones8 = _band(81, 81, -1, [1.0, 1.0, 1.0])
    bones8[:, 0] = 0.0
    bones8[:, 80] = 0.0
    mats["BONES8"] = bones8
    gycor = np.zeros((84, 2), np.float32)
    gycor[83, 0] = 4.0 * np.float32(MU)   # clipped B10M1_8 col 83: colsum 1
    gycor[83, 1] = 12.0 * np.float32(MU)  # 3 channels summed, for gys8
    mats["GYCOR"] = gycor
    return {k: np.ascontiguousarray(v, np.float32) for k, v in mats.items()}


MAT_SPECS = {
    "GYCOR": [84, 2],
    "BV": [128, 124], "B121": [124, 122], "B121N": [124, 122],
    "B10M1": [124, 122], "B10M1X2": [124, 122], "BONES": [120, 119],
    "BV8": [88, 86], "B121_8": [86, 84], "B121N_8": [86, 84],
    "B10M1_8": [86, 84], "B10M1X2_8": [86, 84], "BONES8": [81, 81],
}
F32R_MATS = ("B121", "B121N", "B10M1", "B10M1X2",
             "B121_8", "B121N_8", "B10M1_8", "B10M1X2_8")
BF16_MATS = ("BONES", "BONES8")

_CACHE = {}


def _build_program():
    if "nc" in _CACHE:
        return _CACHE["nc"]
    import concourse.bass as bass
    import concourse.mybir as mybir
    from concourse.tile import TileContext

    f32 = mybir.dt.float32
    f32r = mybir.dt.float32r
    bf16 = mybir.dt.bfloat16
    u8 = mybir.dt.uint8
    Alu = mybir.AluOpType

    g = _gauss5()
    R10G = float(g[1] / g[0])
    R20G = float(g[2] / g[0])

    nc = bass.Bass()

    xm = nc.declare_dram_parameter("xm", [B * 3, 128, W], f32, isOutput=False)
    x8 = nc.declare_dram_parameter("x8", [3, 88, W], f32, isOutput=False)
    mat_d = {k: nc.declare_dram_parameter(k, v, f32, isOutput=False)
             for k, v in MAT_SPECS.items()}
    outm = nc.declare_dram_parameter("outm", [B, SLAB, W], f32, isOutput=True)
    out8 = nc.declare_dram_parameter("out8", [B8_ROWS, W], f32, isOutput=True)

    with TileContext(nc) as tc:
        with (
            tc.tile_pool(name="consts", bufs=1) as cpool,
            tc.tile_pool(name="gmp", bufs=1) as gmpool,
            tc.tile_pool(name="mskp", bufs=1) as mskpool,
            tc.tile_pool(name="b8p", bufs=1) as b8pool,
            tc.tile_pool(name="dram", bufs=1, space="DRAM") as dpool,
        ):
            # ---- constants ------------------------------------------------
            mt = {}
            for name, shp in MAT_SPECS.items():
                t = cpool.tile(shp, f32, tag=name)
                nc.sync.dma_start(out=t[:], in_=mat_d[name][:])
                if name in F32R_MATS:
                    tr = cpool.tile(shp, f32r, tag=name + "r")
                    nc.scalar.copy(out=tr[:], in_=t[:])
                    mt[name] = tr
                elif name in BF16_MATS:
                    tb = cpool.tile(shp, bf16, tag=name + "b")
                    nc.scalar.copy(out=tb[:], in_=t[:])
                    mt[name] = tb
                else:
                    mt[name] = t
            bias = cpool.tile([128, 1], f32, tag="bias")
            nc.vector.memset(bias[:], -MU)
            muhi_c = cpool.tile([128, 1], f32, tag="muhi_c")
            nc.vector.memset(muhi_c[:], MU_HI)
            mulo_c = cpool.tile([128, 1], f32, tag="mulo_c")
            nc.vector.memset(mulo_c[:], MU_LO)

            # gm composite: [122, 8, 1030] f32; image j plane, data col 3+w,
            # 3 margin cols each side (needed for shifted compare reads)
            gm_all = gmpool.tile([122, B, 1030], f32, tag="gm_all")
            # B8 gm: [85, 1026] f32; row p <-> img 940+p, data col 1+w
            gm8 = b8pool.tile([85, 1026], f32, tag="gm8")
            nc.vector.memset(gm8[:], 0.0)
            # thin-frame mask composites (u8), 1-col margins per plane
            mco = {t: mskpool.tile([120, B, 1026], u8, tag=f"mc_{t}",
                                   name=f"mc_{t}")
                   for t in ("c0", "c2", "sm")}
            for t in ("c0", "c2", "sm"):
                nc.vector.memset(mco[t][:, :, 0:1], 0)
                nc.vector.memset(mco[t][:, :, W + 1:W + 2], 0)
            m8 = {t: b8pool.tile([81, 1024], u8, tag=f"m8_{t}", name=f"m8_{t}")
                  for t in ("c0", "c2", "sm")}
            ce8 = b8pool.tile([81, 1026], f32, tag="ce8")
            a2a_in = dpool.tile([B * 81, W], u8, tag="a2a_in")
            a2a_out = dpool.tile([B * 81, W], u8, tag="a2a_out")

            # =========== conv phase ========================================
            with (
                tc.tile_pool(name="xin", bufs=2) as xpool,
                tc.tile_pool(name="hbt", bufs=2) as hbpool,
                tc.tile_pool(name="blt", bufs=2) as blpool,
                tc.tile_pool(name="sq", bufs=2) as sqpool,
                tc.tile_pool(name="gsum", bufs=2) as gsumpool,
                tc.tile_pool(name="mskt", bufs=2) as msktpool,
                tc.tile_pool(name="psA", bufs=2, space="PSUM") as psA,
                tc.tile_pool(name="psB", bufs=1, space="PSUM") as psB,
            ):
                def conv_channel(xt, n_in, n_bl, n_gxy, bv, b121, b121n,
                                 b10m1, b10m1x2, hb_eng, gm_dst, mag_c,
                                 gxs, gys, c, sqy_bias=None):
                    """One channel: h-blur, fp32 v-blur, split-f32r sobel,
                    magnitude.  gm_dst: AP for this image's gm slice rows
                    [0:n_gxy]; mag_c: list collecting per-channel mag tiles."""
                    # h-blur, symmetric: h2 = t1 + (g1/g0) t2 + (g2/g0) x2
                    # where t1 = x[-2]+x[2], t2 = x[-1]+x[1].  Pool does the
                    # two adds, DVE the two fused madds.
                    h1 = hbpool.tile([n_in, W], f32, tag="h1")
                    h2 = hbpool.tile([n_in, W], f32, tag="h2")
                    t1 = hbpool.tile([n_in, W], f32, tag="t1")
                    nc.gpsimd.tensor_tensor(out=t1[:], in0=xt[:, 0:W],
                                            in1=xt[:, 4:W + 4], op=Alu.add)
                    nc.gpsimd.tensor_tensor(out=h1[:], in0=xt[:, 1:W + 1],
                                            in1=xt[:, 3:W + 3], op=Alu.add)
                    nc.vector.scalar_tensor_tensor(
                        out=h2[:], in0=h1[:], scalar=R10G, in1=t1[:],
                        op0=Alu.mult, op1=Alu.add)
                    nc.vector.scalar_tensor_tensor(
                        out=h2[:], in0=xt[:, 2:W + 2], scalar=R20G,
                        in1=h2[:], op0=Alu.mult, op1=Alu.add)
                    # v-blur: exact fp32 matmul -> PSUM
                    bl = psA.tile([n_bl, W], f32, tag="bl")
                    for lo in (0, 512):
                        nc.tensor.matmul(out=bl[:, lo:lo + 512],
                                         lhsT=bv[0:n_in, 0:n_bl],
                                         rhs=h2[:, lo:lo + 512],
                                         start=True, stop=True)
                    # center + split into f32r hi/lo with -MU margins
                    bhi = blpool.tile([n_bl, W + 2], f32r, tag="bhi")
                    blo = blpool.tile([n_bl, W + 2], f32r, tag="blo")
                    for mcol, dsts in ((muhi_c, bhi), (mulo_c, blo)):
                        for cs in (slice(0, 1), slice(W + 1, W + 2)):
                            nc.vector.tensor_scalar(
                                out=dsts[:, cs], in0=mcol[0:n_bl, 0:1],
                                scalar1=1.0, scalar2=None, op0=Alu.mult)
                    nc.scalar.add(out=bhi[:, 1:W + 1], in_=bl[:],
                                  add=bias[0:n_bl, 0:1])
                    nc.vector.scalar_tensor_tensor(
                        out=blo[:, 1:W + 1], in0=bl[:], scalar=-MU,
                        in1=bhi[:, 1:W + 1], op0=Alu.add, op1=Alu.subtract)
                    # sobel: 20 f32r matmuls (hi+lo accumulate in PSUM)
                    gx = psB.tile([n_gxy, W], f32, tag="gx")
                    gy = psB.tile([n_gxy, W], f32, tag="gy")
                    for lo in (0, 512):
                        first = True
                        for src in (bhi, blo):
                            sm_ = src[:, 0 + lo:512 + lo]
                            sc = src[:, 1 + lo:513 + lo]
                            sp = src[:, 2 + lo:514 + lo]
                            nc.tensor.matmul(out=gx[:, lo:lo + 512],
                                             lhsT=b121[0:n_bl, 0:n_gxy],
                                             rhs=sm_, start=first, stop=False)
                            nc.tensor.matmul(out=gx[:, lo:lo + 512],
                                             lhsT=b121n[0:n_bl, 0:n_gxy],
                                             rhs=sp, start=False,
                                             stop=(src is blo))
                            nc.tensor.matmul(out=gy[:, lo:lo + 512],
                                             lhsT=b10m1[0:n_bl, 0:n_gxy],
                                             rhs=sp, start=first, stop=False)
                            nc.tensor.matmul(out=gy[:, lo:lo + 512],
                                             lhsT=b10m1x2[0:n_bl, 0:n_gxy],
                                             rhs=sc, start=False, stop=False)
                            nc.tensor.matmul(out=gy[:, lo:lo + 512],
                                             lhsT=b10m1[0:n_bl, 0:n_gxy],
                                             rhs=sm_, start=False,
                                             stop=(src is blo))
                            first = False
                    # magnitude: mag_c[c] = sqrt(gx^2 + gy^2)
                    sqx = sqpool.tile([n_gxy, W], f32, tag="sqx")
                    sqy = sqpool.tile([n_gxy, W], f32, tag="sqy")
                    nc.scalar.square(out=sqx[:], in_=gx[:])
                    if sqy_bias is None:
                        nc.scalar.square(out=sqy[:], in_=gy[:])
                    else:
                        nc.scalar.activation(
                            out=sqy[:], in_=gy[:],
                            func=mybir.ActivationFunctionType.Square,
                            bias=sqy_bias)
                    nc.vector.tensor_tensor(out=sqx[:], in0=sqx[:],
                                            in1=sqy[:], op=Alu.add)
                    mg = sqpool.tile([n_gxy, W], f32, tag=f"mag{c}")
                    nc.scalar.sqrt(out=mg[:], in_=sqx[:])
                    mag_c.append(mg)
                    # orientation sums
                    if c == 0:
                        nc.scalar.copy(out=gxs[0:n_gxy, :], in_=gx[:])
                        nc.scalar.copy(out=gys[0:n_gxy, :], in_=gy[:])
                    else:
                        nc.vector.tensor_tensor(out=gxs[0:n_gxy, :],
                                                in0=gxs[0:n_gxy, :],
                                                in1=gx[:], op=Alu.add)
                        nc.vector.tensor_tensor(out=gys[0:n_gxy, :],
                                                in0=gys[0:n_gxy, :],
                                                in1=gy[:], op=Alu.add)
                    if c == 2:
                        # gm = mag0 + mag1 + mag2; single writer of gm slice
                        t01 = sqpool.tile([n_gxy, W], f32, tag="sqy")
                        nc.vector.tensor_tensor(out=t01[:], in0=mag_c[0][:],
                                                in1=mag_c[1][:], op=Alu.add)
                        nc.gpsimd.tensor_tensor(out=gm_dst, in0=t01[:],
                                                in1=mag_c[2][:], op=Alu.add)

                def make_masks(gxs, gys, n, shift, n_thin, dst, dst_slices):
                    """u8 masks at conv frame [0:n]; DMA rows
                    [shift:shift+n_thin] into dst[t] slices."""
                    a2 = sqpool.tile([n, W], f32, tag="sqx")
                    b2 = sqpool.tile([n, W], f32, tag="sqy")
                    nc.scalar.square(out=a2[:, :], in_=gxs[0:n, :])
                    nc.scalar.square(out=b2[:, :], in_=gys[0:n, :])
                    tmp = {t: msktpool.tile([n, W], u8, tag=f"t{t}", name=f"t{t}")
                           for t in ("c0", "c2", "sm")}
                    nc.vector.scalar_tensor_tensor(
                        out=tmp["c0"][:], in0=a2[:], scalar=T22SQ,
                        in1=b2[:], op0=Alu.mult, op1=Alu.is_gt)
                    nc.vector.scalar_tensor_tensor(
                        out=tmp["c2"][:], in0=b2[:], scalar=T22SQ,
                        in1=a2[:], op0=Alu.mult, op1=Alu.is_gt)
                    ab = sqpool.tile([n, W], f32, tag="mag0")
                    nc.gpsimd.tensor_tensor(out=ab[:], in0=gxs[0:n, :],
                                            in1=gys[0:n, :], op=Alu.mult)
                    nc.vector.tensor_scalar(out=tmp["sm"][:], in0=ab[:],
                                            scalar1=0.0, scalar2=None,
                                            op0=Alu.is_ge)
                    for t in ("c0", "c2", "sm"):
                        nc.sync.dma_start(
                            out=dst_slices(dst[t]),
                            in_=tmp[t][shift:shift + n_thin, :])

                # ---- B8 block first (feeds the collective) ---------------
                gxs8 = gsumpool.tile([84, W], f32, tag="gxs")
                gys8 = gsumpool.tile([84, W], f32, tag="gys")
                mag8 = []
                for c in range(3):
                    xt = xpool.tile([88, W + 4], f32, tag="x")
                    nc.vector.memset(xt[:, 0:2], 0.0)
                    nc.vector.memset(xt[:, W + 2:W + 4], 0.0)
                    nc.sync.dma_start(out=xt[:, 2:W + 2], in_=x8[c])
                    conv_channel(xt, 88, 86, 84, mt["BV8"], mt["B121_8"],
                                 mt["B121N_8"], mt["B10M1_8"],
                                 mt["B10M1X2_8"],
                                 nc.gpsimd if c != 2 else nc.vector,
                                 gm8[0:84, 1:W + 1], mag8, gxs8, gys8, c,
                                 sqy_bias=mt["GYCOR"][0:84, 0:1])
                gys8c = gsumpool.tile([84, W], f32, tag="gys8c", bufs=1)
                nc.scalar.add(out=gys8c[:], in_=gys8[:],
                              add=mt["GYCOR"][0:84, 1:2])
                make_masks(gxs8, gys8c, 84, 3, 81, m8, lambda t: t[:])

                # D maps: realign gm8 rows and compare in 8 directions
                ce8d = msktpool.tile([81, 1026], f32, tag="dn8", bufs=1)
                up8d = msktpool.tile([81, 1026], f32, tag="up8", bufs=1)
                nc.sync.dma_start(out=ce8[:], in_=gm8[3:84, :])
                nc.sync.dma_start(out=ce8d[:], in_=gm8[4:85, :])
                nc.sync.dma_start(out=up8d[:], in_=gm8[2:83, :])
                d8 = msktpool.tile([81, B, 1024], u8, tag="d8", bufs=1)
                for d, (dr, dc) in DELTAS.items():
                    src = {0: ce8, 1: ce8d, -1: up8d}[dr]
                    nc.vector.tensor_tensor(out=d8[:, d, :],
                                            in0=ce8[:, 1:W + 1],
                                            in1=src[:, 1 + dc:W + 1 + dc],
                                            op=Alu.is_gt)
                for d in range(B):
                    nc.sync.dma_start(out=a2a_in[81 * d:81 * (d + 1), :],
                                      in_=d8[:, d, :])
                nc.gpsimd.collective_compute(
                    "AllToAll", Alu.bypass,
                    replica_groups=[list(range(NC))],
                    ins=[a2a_in.opt()], outs=[a2a_out.opt()])

                # ---- main slab: 8 images x 3 channels --------------------
                for j in range(B):
                    nc.vector.memset(gm_all[:, j, 0:3], 0.0)
                    nc.vector.memset(gm_all[:, j, W + 3:W + 6], 0.0)
                    gxs = gsumpool.tile([122, W], f32, tag="gxs")
                    gys = gsumpool.tile([122, W], f32, tag="gys")
                    mag_c = []
                    for c in range(3):
                        xt = xpool.tile([128, W + 4], f32, tag="x")
                        nc.vector.memset(xt[:, 0:2], 0.0)
                        nc.vector.memset(xt[:, W + 2:W + 4], 0.0)
                        nc.sync.dma_start(out=xt[:, 2:W + 2], in_=xm[3 * j + c])
                        hb = nc.gpsimd if c != 2 else nc.vector
                        conv_channel(xt, 128, 124, 122, mt["BV"], mt["B121"],
                                     mt["B121N"], mt["B10M1"], mt["B10M1X2"],
                                     hb, gm_all[0:122, j, 3:W + 3], mag_c,
                                     gxs, gys, c)
                    make_masks(gxs, gys, 122, 1, 120, mco,
                               lambda t, j=j: t[:, j, 1:W + 1])

            # =========== NMS phase =========================================
            with (
                tc.tile_pool(name="cep", bufs=1) as cepool,
                tc.tile_pool(name="cb", bufs=2) as cbpool,
                tc.tile_pool(name="pb", bufs=1) as pbpool,
                tc.tile_pool(name="tail", bufs=2) as tlpool,
                tc.tile_pool(name="otp", bufs=1) as otpool,
                tc.tile_pool(name="psC", bufs=2, space="PSUM") as psC,
            ):
                ce121 = cepool.tile([121, B, 1030], f32, tag="ce121")
                nc.sync.dma_start(out=ce121[:], in_=gm_all[1:122, :, :])

                def nms_tail(b, pcomp, half):
                    """psel/strong/q/mh/mp/ot/out for output image b.
                    pcomp: P composite [120,4,514] covering
                    w in [base-1 .. base+512]."""
                    base = 512 * half
                    psel = tlpool.tile([120, 514], bf16, tag="psel")
                    nc.scalar.copy(out=psel[:], in_=pcomp[:, 3, :])
                    for t, k in (("sm", 1), ("c0", 0), ("c2", 2)):
                        nc.vector.copy_predicated(
                            out=psel[:], mask=mco[t][:, b, base:base + 514],
                            data=pcomp[:, k, :])
                    hi1 = tlpool.tile([120, 514], bf16, tag="hi1", bufs=1)
                    nc.vector.tensor_scalar(
                        out=hi1[:], in0=ce121[0:120, b, base + 2:base + 516],
                        scalar1=HIGH_T, scalar2=None, op0=Alu.is_gt)
                    strong = tlpool.tile([120, 514], bf16, tag="strong")
                    nc.gpsimd.tensor_tensor(out=strong[:], in0=hi1[:],
                                            in1=psel[:], op=Alu.mult)
                    q1 = tlpool.tile([120, 512], bf16, tag="q1", bufs=1)
                    nc.vector.tensor_scalar(
                        out=q1[:], in0=ce121[0:120, b, base + 3:base + 515],
                        scalar1=LOW_T, scalar2=None, op0=Alu.is_ge)
                    q = tlpool.tile([120, 512], bf16, tag="q")
                    nc.gpsimd.tensor_tensor(out=q[:], in0=q1[:],
                                            in1=psel[:, 1:513],
                                            op=Alu.mult)
                    mh = tlpool.tile([120, 512], bf16, tag="mh")
                    nc.vector.tensor_tensor(out=mh[:], in0=strong[:, 0:512],
                                            in1=strong[:, 2:514], op=Alu.add)
                    nc.vector.tensor_tensor(out=mh[:], in0=mh[:],
                                            in1=strong[:, 1:513], op=Alu.add)
                    mp = psC.tile([119, 512], f32, tag="mp")
                    nc.tensor.matmul(out=mp[:], lhsT=mt["BONES"][0:120, 0:119],
                                     rhs=mh[:], start=True, stop=True)
                    ot = otpool.tile([119, 512], f32, tag="ot")
                    nc.vector.scalar_tensor_tensor(
                        out=ot[:], in0=mp[:], scalar=0.5, in1=q[0:119, :],
                        op0=Alu.is_ge, op1=Alu.logical_and)
                    if half == 0:
                        nc.vector.memset(ot[:, 0:1], 0.0)
                    else:
                        nc.vector.memset(ot[:, 511:512], 0.0)
                    nc.sync.dma_start(out=outm[b][:, base:base + 512],
                                      in_=ot[1:119, :])

                # ---- B8 NMS first: inputs (a2a_out, ce8, m8) are ready
                dr8 = cbpool.tile([81, B, 1024], u8, tag="dr8", bufs=1)
                for d in range(B):
                    nc.sync.dma_start(out=dr8[:, d, :],
                                      in_=a2a_out[81 * d:81 * (d + 1), :])
                p8 = cbpool.tile([81, 4, 1024], u8, tag="p8", bufs=1)
                nc.vector.tensor_tensor(out=p8[:], in0=dr8[:, 0:4, :],
                                        in1=dr8[:, 4:8, :],
                                        op=Alu.logical_and)
                psel8 = cbpool.tile([81, 1024], u8, tag="psel8", bufs=1)
                nc.scalar.copy(out=psel8[:], in_=p8[:, 3, :])
                for t, k in (("sm", 1), ("c0", 0), ("c2", 2)):
                    nc.vector.copy_predicated(out=psel8[:], mask=m8[t][:],
                                              data=p8[:, k, :])
                strong8 = cbpool.tile([81, 1026], bf16, tag="strong8", bufs=1)
                nc.vector.memset(strong8[:, 0:1], 0.0)
                nc.vector.memset(strong8[:, W + 1:W + 2], 0.0)
                hi8 = cbpool.tile([81, 1024], bf16, tag="hi8", bufs=1)
                nc.vector.tensor_scalar(
                    out=hi8[:], in0=ce8[:, 1:W + 1], scalar1=HIGH_T,
                    scalar2=None, op0=Alu.is_gt)
                ps8b = cbpool.tile([81, 1024], bf16, tag="ps8b", bufs=1)
                nc.scalar.copy(out=ps8b[:], in_=psel8[:])
                nc.gpsimd.tensor_tensor(out=strong8[:, 1:W + 1], in0=hi8[:],
                                        in1=ps8b[:], op=Alu.mult)
                q8 = cbpool.tile([81, 1024], u8, tag="q8", bufs=1)
                nc.vector.scalar_tensor_tensor(
                    out=q8[:], in0=ce8[:, 1:W + 1], scalar=LOW_T,
                    in1=psel8[:], op0=Alu.is_ge, op1=Alu.logical_and)
                mh8 = cbpool.tile([81, 1024], bf16, tag="mh8", bufs=1)
                nc.vector.tensor_tensor(out=mh8[:], in0=strong8[:, 0:W],
                                        in1=strong8[:, 2:W + 2], op=Alu.add)
                nc.vector.tensor_tensor(out=mh8[:], in0=mh8[:],
                                        in1=strong8[:, 1:W + 1], op=Alu.add)
                mp8 = psC.tile([81, 1024], f32, tag="mp8", bufs=1)
                for lo in (0, 512):
                    nc.tensor.matmul(out=mp8[:, lo:lo + 512],
                                     lhsT=mt["BONES8"][0:81, 0:81],
                                     rhs=mh8[:, lo:lo + 512],
                                     start=True, stop=True)
                ot8 = cbpool.tile([81, 1024], f32, tag="ot8", bufs=1)
                nc.vector.scalar_tensor_tensor(
                    out=ot8[:], in0=mp8[:], scalar=0.5, in1=q8[:],
                    op0=Alu.is_ge, op1=Alu.logical_and)
                nc.vector.memset(ot8[:, 0:1], 0.0)
                nc.vector.memset(ot8[:, W - 1:W], 0.0)
                nc.sync.dma_start(out=out8[:], in_=ot8[1:81, :])

                for half in (0, 1):
                    base = 512 * half
                    for b in (0, 1, 2, 3):
                        dr, dc = DELTAS[b]
                        # C col i <-> w = base-2+i (516 cols); col(w) = 3+w
                        s0 = base + 1
                        if b == 0:
                            cth = cbpool.tile([120, B, 516], bf16, tag="c0t",
                                              bufs=1)
                            nc.vector.tensor_tensor(
                                out=cth[:],
                                in0=ce121[0:120, :, s0:s0 + 516],
                                in1=ce121[0:120, :, s0 + dc:s0 + dc + 516],
                                op=Alu.is_gt)
                            cx_low = cth  # dr=0: same rows for shifted view
                        else:
                            df = cbpool.tile([121, B, 516], f32, tag="df",
                                             bufs=1)
                            nc.gpsimd.tensor_tensor(
                                out=df[:],
                                in0=gm_all[0:121, :, s0:s0 + 516],
                                in1=ce121[0:121, :, s0 + dc:s0 + dc + 516],
                                op=Alu.subtract)
                            cx = cbpool.tile([121, B, 516], bf16, tag="cx",
                                             bufs=2)
                            nc.vector.tensor_scalar(
                                out=cx[:], in0=df[:], scalar1=0.0,
                                scalar2=None, op0=Alu.is_gt)
                            cth = cbpool.tile([120, B, 516], bf16, tag="cth",
                                              bufs=1)
                            nc.sync.dma_start(out=cth[:], in_=cx[1:121, :, :])
                            cx_low = cx  # rows 0:120 = thin p-1 view
                        # P_b: planes k AND k+4 on w [base-1 .. base+512]
                        pb = pbpool.tile([120, 4, 514], bf16, tag="pb")
                        nc.vector.tensor_tensor(
                            out=pb[:], in0=cth[:, 0:4, 1:515],
                            in1=cth[:, 4:8, 1:515], op=Alu.logical_and)
                        nms_tail(b, pb, half)
                        # P_{b+4} = NOT C_b(p-dr, w-dc) pairwise: sum==0
                        ss = 1 - dc
                        sb = pbpool.tile([120, 4, 514], bf16, tag="sb")
                        nc.vector.tensor_tensor(
                            out=sb[:],
                            in0=cx_low[0:120, 0:4, ss:ss + 514],
                            in1=cx_low[0:120, 4:8, ss:ss + 514],
                            op=Alu.add)
                        pb4 = pbpool.tile([120, 4, 514], bf16, tag="pb4")
                        nc.vector.tensor_scalar(
                            out=pb4[:], in0=sb[:], scalar1=0.0, scalar2=None,
                            op0=Alu.is_equal)
                        nms_tail(b + 4, pb4, half)

    _legalize_waits(nc)
    _CACHE["nc"] = nc
    return nc


def _legalize_waits(nc):
    """Hoist embedded waits of multi-wait instructions into NoOps (several
    ISA encodings hold only one embedded sync-wait)."""
    import concourse.mybir as mybir
    n = 0
    for f in nc.m.functions:
        for blk in f.blocks:
            out = []
            for ins in blk.instructions:
                si = ins.sync_info
                if (si is not None and si.on_wait is not None
                        and len(si.on_wait) > 1):
                    for w in si.on_wait:
                        nop = mybir.InstNoOp(
                            name=f"WFIX-{n}", engine=ins.engine,
                            sync_info=mybir.SyncInfo(on_wait=[w],
                                                     on_update=[]))
                        n += 1
                        out.append(nop)
                    ins.sync_info = mybir.SyncInfo(
                        on_wait=[],
                        on_update=list(si.on_update or []))
                out.append(ins)
            blk.instructions = out


def _in_maps(img):
    img = np.ascontiguousarray(img, dtype=np.float32)
    pad = np.zeros((B, 3, 5, W), np.float32)
    imgp = np.concatenate([pad, img], axis=2)  # rows shifted by +5
    maps = []
    for i in range(NC):
        r0 = SLAB * i
        xm_i = imgp[:, :, r0:r0 + 128, :].reshape(B * 3, 128, W)
        x8_i = img[i, :, B8_START - 8:, :]
        m = {"xm": np.ascontiguousarray(xm_i),
             "x8": np.ascontiguousarray(x8_i)}
        m.update(_const_mats(i))
        maps.append(m)
    return maps


def kernel(img, gauss_h=None, gauss_v=None, sobel_h=None, sobel_v=None,
           dir_f=None, connect_f=None, _want_time=False):
    from concourse.bass_utils import run_bass_kernel_spmd
    nc = _build_program()
    maps = _in_maps(np.asarray(img))
    res = run_bass_kernel_spmd(nc, maps, list(range(NC)), trace=_want_time)
    out = np.zeros((B, 1, H, W), np.float32)
    for i in range(NC):
        r = res.results[i]
        out[:, 0, SLAB * i:SLAB * (i + 1), :] = r["outm"]
        out[i, 0, B8_START:, :] = r["out8"]
    if _want_time:
        return out, res
    return out


# revision 32
# speedup vs baseline: 1.2513x; 1.0378x over previous
"""Canny edge detector on 8 Trainium2 NeuronCores (Bass/Tile) — v2.

Sharding: row slabs (see baseline docstring for why: the reference's flat
gather cross-wires images, so every output pixel needs all 8 images' gm).

v2 changes vs baseline:
- Sobel matmuls run in fp32r (4x PE throughput) with exact precision: the
  blurred field is mean-centered (-MU) and split into hi = f32r(bl') and
  lo = bl' - hi; sobel weights are small integers (exact in fp32r), so
  accumulating hi+lo in PSUM reproduces the f32 result to ~1e-6.
- B8 band computed FIRST; only its 8 direction-compare bitmaps (u8) are
  exchanged via ONE AllToAll (663KB vs 5.3MB AllGather+AllToAll in f32),
  fully overlapped with the main conv.
- gm for all 8 images lives in one 3D composite tile -> NMS compares are
  one instruction per (direction, half) over all images.
- Directions 4..7 reuse directions 0..3: C_{b+4}(p) = NOT C_b(p - delta)
  (exact up to f32 ties, which do not occur for this data).
- Elementwise work spread across DVE/Pool/Act per a makespan balance.
"""

import os

os.environ.setdefault("BY_DEFAULT_DISABLE_SUBTILE_DEPS", "1")

import numpy as np

H = 1024
W = 1024
B = 8
NC = 8
SLAB = 118
B8_START = SLAB * NC          # 944
B8_ROWS = H - B8_START        # 80
LOW_T, HIGH_T = 2.5, 5.0
T22SQ = float(np.float32(np.tan(np.pi / 8.0)) ** 2)
MU = 3.0807319                # E[bl] for uniform input; exactness not needed

DELTAS = {0: (0, 1), 1: (1, 1), 2: (1, 0), 3: (1, -1),
          4: (0, -1), 5: (-1, -1), 6: (-1, 0), 7: (-1, 1)}


def _fp32r_round(v):
    u = np.asarray(v, np.float32).reshape(1).view(np.uint32)
    r = ((u >> 12) & 1) + 0x07FF
    return float(((u + r) & ~np.uint32(0xFFF)).view(np.float32)[0])


MU_HI = _fp32r_round(-MU)                       # hi-margin value (= f32r(-MU))
MU_LO = _fp32r_round(np.float32(-MU) - np.float32(MU_HI))


def _gauss5():
    n = np.arange(5, dtype=np.float32) - 2.0
    return np.exp(-0.5 * n * n).astype(np.float32)


def _band(n_in, n_out, offset, taps):
    m_ = np.zeros((n_in, n_out), np.float32)
    for mm in range(n_out):
        for t, w in enumerate(taps):
            k = mm + offset + t
            if 0 <= k < n_in:
                m_[k, mm] = w
    return m_


def _const_mats(core):
    g = _gauss5()
    g0 = float(g[0])
    mats = {}
    mats["BV"] = _band(128, 124, 0, (g0 * g).tolist())
    b121 = _band(124, 122, 0, [1.0, 2.0, 1.0])
    b10m1 = _band(124, 122, 0, [1.0, 0.0, -1.0])
    if core == 0:  # img rows -2,-1 must yield gm=0 (zero-pad semantics)
        b121[:, 0:2] = 0.0
        b10m1[:, 0:2] = 0.0
    mats["B121"] = b121
    mats["B121N"] = -b121
    mats["B10M1"] = b10m1
    mats["B10M1X2"] = 2.0 * b10m1
    bones = _band(120, 119, -1, [1.0, 1.0, 1.0])
    bones[:, 0] = 0.0
    if core == 0:
        bones[:, 1] = 0.0
    mats["BONES"] = bones
    mats["BV8"] = _band(88, 86, 0, (g0 * g).tolist())
    b121_8 = _band(86, 84, 1, [1.0, 2.0, 1.0])
    b10m1_8 = _band(86, 84, 1, [1.0, 0.0, -1.0])
    mats["B121_8"] = b121_8
    mats["B121N_8"] = -b121_8
    mats["B10M1_8"] = b10m1_8
    mats["B10M1X2_8"] = 2.0 * b10m1_8
    bones8 = _band(81, 81, -1, [1.0, 1.0, 1.0])
    bones8[:, 0] = 0.0
    bones8[:, 80] = 0.0
    mats["BONES8"] = bones8
    gycor = np.zeros((84, 2), np.float32)
    gycor[83, 0] = 4.0 * np.float32(MU)   # clipped B10M1_8 col 83: colsum 1
    gycor[83, 1] = 12.0 * np.float32(MU)  # 3 channels summed, for gys8
    mats["GYCOR"] = gycor
    return {k: np.ascontiguousarray(v, np.float32) for k, v in mats.items()}


MAT_SPECS = {
    "GYCOR": [84, 2],
    "BV": [128, 124], "B121": [124, 122], "B121N": [124, 122],
    "B10M1": [124, 122], "B10M1X2": [124, 122], "BONES": [120, 119],
    "BV8": [88, 86], "B121_8": [86, 84], "B121N_8": [86, 84],
    "B10M1_8": [86, 84], "B10M1X2_8": [86, 84], "BONES8": [81, 81],
}
F32R_MATS = ("B121", "B121N", "B10M1", "B10M1X2",
             "B121_8", "B121N_8", "B10M1_8", "B10M1X2_8")
BF16_MATS = ("BONES", "BONES8")

_CACHE = {}


def _build_program():
    if "nc" in _CACHE:
        return _CACHE["nc"]
    import concourse.bass as bass
    import concourse.mybir as mybir
    from concourse.tile import TileContext

    f32 = mybir.dt.float32
    f32r = mybir.dt.float32r
    bf16 = mybir.dt.bfloat16
    u8 = mybir.dt.uint8
    Alu = mybir.AluOpType

    g = _gauss5()
    R10G = float(g[1] / g[0])
    R20G = float(g[2] / g[0])

    nc = bass.Bass()

    xm = nc.declare_dram_parameter("xm", [B * 3, 128, W], f32, isOutput=False)
    x8 = nc.declare_dram_parameter("x8", [3, 88, W], f32, isOutput=False)
    mat_d = {k: nc.declare_dram_parameter(k, v, f32, isOutput=False)
             for k, v in MAT_SPECS.items()}
    outm = nc.declare_dram_parameter("outm", [B, SLAB, W], f32, isOutput=True)
    out8 = nc.declare_dram_parameter("out8", [B8_ROWS, W], f32, isOutput=True)

    with TileContext(nc) as tc:
        with (
            tc.tile_pool(name="consts", bufs=1) as cpool,
            tc.tile_pool(name="gmp", bufs=1) as gmpool,
            tc.tile_pool(name="mskp", bufs=1) as mskpool,
            tc.tile_pool(name="b8p", bufs=1) as b8pool,
            tc.tile_pool(name="dram", bufs=1, space="DRAM") as dpool,
        ):
            # ---- constants ------------------------------------------------
            mt = {}
            for name, shp in MAT_SPECS.items():
                t = cpool.tile(shp, f32, tag=name)
                nc.sync.dma_start(out=t[:], in_=mat_d[name][:])
                if name in F32R_MATS:
                    tr = cpool.tile(shp, f32r, tag=name + "r")
                    nc.scalar.copy(out=tr[:], in_=t[:])
                    mt[name] = tr
                elif name in BF16_MATS:
                    tb = cpool.tile(shp, bf16, tag=name + "b")
                    nc.scalar.copy(out=tb[:], in_=t[:])
                    mt[name] = tb
                else:
                    mt[name] = t
            bias = cpool.tile([128, 1], f32, tag="bias")
            nc.vector.memset(bias[:], -MU)
            muhi_c = cpool.tile([128, 1], f32, tag="muhi_c")
            nc.vector.memset(muhi_c[:], MU_HI)
            mulo_c = cpool.tile([128, 1], f32, tag="mulo_c")
            nc.vector.memset(mulo_c[:], MU_LO)

            # gm composite: [122, 8, 1030] f32; image j plane, data col 3+w,
            # 3 margin cols each side (needed for shifted compare reads)
            gm_all = gmpool.tile([122, B, 1030], f32, tag="gm_all")
            # B8 gm: [85, 1026] f32; row p <-> img 940+p, data col 1+w
            gm8 = b8pool.tile([85, 1026], f32, tag="gm8")
            nc.vector.memset(gm8[:], 0.0)
            # thin-frame mask composites (u8), 1-col margins per plane
            mco = {t: mskpool.tile([120, B, 1026], u8, tag=f"mc_{t}",
                                   name=f"mc_{t}")
                   for t in ("c0", "c2", "sm")}
            for t in ("c0", "c2", "sm"):
                nc.vector.memset(mco[t][:, :, 0:1], 0)
                nc.vector.memset(mco[t][:, :, W + 1:W + 2], 0)
            m8 = {t: b8pool.tile([81, 1024], u8, tag=f"m8_{t}", name=f"m8_{t}")
                  for t in ("c0", "c2", "sm")}
            ce8 = b8pool.tile([81, 1026], f32, tag="ce8")
            a2a_in = dpool.tile([B * 81, W], u8, tag="a2a_in")
            a2a_out = dpool.tile([B * 81, W], u8, tag="a2a_out")

            # =========== conv phase ========================================
            with (
                tc.tile_pool(name="xin", bufs=3) as xpool,
                tc.tile_pool(name="hbt", bufs=2) as hbpool,
                tc.tile_pool(name="blt", bufs=2) as blpool,
                tc.tile_pool(name="sq", bufs=2) as sqpool,
                tc.tile_pool(name="gsum", bufs=2) as gsumpool,
                tc.tile_pool(name="mskt", bufs=2) as msktpool,
                tc.tile_pool(name="psA", bufs=2, space="PSUM") as psA,
                tc.tile_pool(name="psB", bufs=1, space="PSUM") as psB,
            ):
                def conv_channel(xt, n_in, n_bl, n_gxy, bv, b121, b121n,
                                 b10m1, b10m1x2, hb_eng, gm_dst, mag_c,
                                 gxs, gys, c, sqy_bias=None):
                    """One channel: h-blur, fp32 v-blur, split-f32r sobel,
                    magnitude.  gm_dst: AP for this image's gm slice rows
                    [0:n_gxy]; mag_c: list collecting per-channel mag tiles."""
                    # h-blur, symmetric: h2 = t1 + (g1/g0) t2 + (g2/g0) x2
                    # where t1 = x[-2]+x[2], t2 = x[-1]+x[1].  Pool does the
                    # two adds, DVE the two fused madds.
                    h1 = hbpool.tile([n_in, W], f32, tag="h1")
                    h2 = hbpool.tile([n_in, W], f32, tag="h2")
                    t1 = hbpool.tile([n_in, W], f32, tag="t1")
                    nc.gpsimd.tensor_tensor(out=t1[:], in0=xt[:, 0:W],
                                            in1=xt[:, 4:W + 4], op=Alu.add)
                    nc.gpsimd.tensor_tensor(out=h1[:], in0=xt[:, 1:W + 1],
                                            in1=xt[:, 3:W + 3], op=Alu.add)
                    nc.vector.scalar_tensor_tensor(
                        out=h2[:], in0=h1[:], scalar=R10G, in1=t1[:],
                        op0=Alu.mult, op1=Alu.add)
                    nc.vector.scalar_tensor_tensor(
                        out=h2[:], in0=xt[:, 2:W + 2], scalar=R20G,
                        in1=h2[:], op0=Alu.mult, op1=Alu.add)
                    # v-blur: exact fp32 matmul -> PSUM
                    bl = psA.tile([n_bl, W], f32, tag="bl")
                    for lo in (0, 512):
                        nc.tensor.matmul(out=bl[:, lo:lo + 512],
                                         lhsT=bv[0:n_in, 0:n_bl],
                                         rhs=h2[:, lo:lo + 512],
                                         start=True, stop=True)
                    # center + split into f32r hi/lo with -MU margins
                    bhi = blpool.tile([n_bl, W + 2], f32r, tag="bhi")
                    blo = blpool.tile([n_bl, W + 2], f32r, tag="blo")
                    for mcol, dsts in ((muhi_c, bhi), (mulo_c, blo)):
                        for cs in (slice(0, 1), slice(W + 1, W + 2)):
                            nc.vector.tensor_scalar(
                                out=dsts[:, cs], in0=mcol[0:n_bl, 0:1],
                                scalar1=1.0, scalar2=None, op0=Alu.mult)
                    nc.scalar.add(out=bhi[:, 1:W + 1], in_=bl[:],
                                  add=bias[0:n_bl, 0:1])
                    nc.vector.scalar_tensor_tensor(
                        out=blo[:, 1:W + 1], in0=bl[:], scalar=-MU,
                        in1=bhi[:, 1:W + 1], op0=Alu.add, op1=Alu.subtract)
                    # sobel: 20 f32r matmuls (hi+lo accumulate in PSUM)
                    gx = psB.tile([n_gxy, W], f32, tag="gx")
                    gy = psB.tile([n_gxy, W], f32, tag="gy")
                    for lo in (0, 512):
                        first = True
                        for src in (bhi, blo):
                            sm_ = src[:, 0 + lo:512 + lo]
                            sc = src[:, 1 + lo:513 + lo]
                            sp = src[:, 2 + lo:514 + lo]
                            nc.tensor.matmul(out=gx[:, lo:lo + 512],
                                             lhsT=b121[0:n_bl, 0:n_gxy],
                                             rhs=sm_, start=first, stop=False)
                            nc.tensor.matmul(out=gx[:, lo:lo + 512],
                                             lhsT=b121n[0:n_bl, 0:n_gxy],
                                             rhs=sp, start=False,
                                             stop=(src is blo))
                            nc.tensor.matmul(out=gy[:, lo:lo + 512],
                                             lhsT=b10m1[0:n_bl, 0:n_gxy],
                                             rhs=sp, start=first, stop=False)
                            nc.tensor.matmul(out=gy[:, lo:lo + 512],
                                             lhsT=b10m1x2[0:n_bl, 0:n_gxy],
                                             rhs=sc, start=False, stop=False)
                            nc.tensor.matmul(out=gy[:, lo:lo + 512],
                                             lhsT=b10m1[0:n_bl, 0:n_gxy],
                                             rhs=sm_, start=False,
                                             stop=(src is blo))
                            first = False
                    # magnitude: mag_c[c] = sqrt(gx^2 + gy^2)
                    sqx = sqpool.tile([n_gxy, W], f32, tag="sqx")
                    sqy = sqpool.tile([n_gxy, W], f32, tag="sqy")
                    nc.scalar.square(out=sqx[:], in_=gx[:])
                    if sqy_bias is None:
                        nc.scalar.square(out=sqy[:], in_=gy[:])
                    else:
                        nc.scalar.activation(
                            out=sqy[:], in_=gy[:],
                            func=mybir.ActivationFunctionType.Square,
                            bias=sqy_bias)
                    nc.vector.tensor_tensor(out=sqx[:], in0=sqx[:],
                                            in1=sqy[:], op=Alu.add)
                    mg = sqpool.tile([n_gxy, W], f32, tag=f"mag{c}", bufs=1 if c == 2 else None)
                    nc.scalar.sqrt(out=mg[:], in_=sqx[:])
                    mag_c.append(mg)
                    # orientation sums
                    if c == 0:
                        nc.scalar.copy(out=gxs[0:n_gxy, :], in_=gx[:])
                        nc.scalar.copy(out=gys[0:n_gxy, :], in_=gy[:])
                    else:
                        nc.vector.tensor_tensor(out=gxs[0:n_gxy, :],
                                                in0=gxs[0:n_gxy, :],
                                                in1=gx[:], op=Alu.add)
                        nc.vector.tensor_tensor(out=gys[0:n_gxy, :],
                                                in0=gys[0:n_gxy, :],
                                                in1=gy[:], op=Alu.add)
                    if c == 2:
                        # gm = mag0 + mag1 + mag2; single writer of gm slice
                        t01 = sqpool.tile([n_gxy, W], f32, tag="sqy")
                        nc.vector.tensor_tensor(out=t01[:], in0=mag_c[0][:],
                                                in1=mag_c[1][:], op=Alu.add)
                        nc.gpsimd.tensor_tensor(out=gm_dst, in0=t01[:],
                                                in1=mag_c[2][:], op=Alu.add)

                def make_masks(gxs, gys, n, shift, n_thin, dst, dst_slices):
                    """u8 masks at conv frame [0:n]; DMA rows
                    [shift:shift+n_thin] into dst[t] slices."""
                    a2 = sqpool.tile([n, W], f32, tag="sqx")
                    b2 = sqpool.tile([n, W], f32, tag="sqy")
                    nc.scalar.square(out=a2[:, :], in_=gxs[0:n, :])
                    nc.scalar.square(out=b2[:, :], in_=gys[0:n, :])
                    tmp = {t: msktpool.tile([n, W], u8, tag=f"t{t}", name=f"t{t}")
                           for t in ("c0", "c2", "sm")}
                    nc.vector.scalar_tensor_tensor(
                        out=tmp["c0"][:], in0=a2[:], scalar=T22SQ,
                        in1=b2[:], op0=Alu.mult, op1=Alu.is_gt)
                    nc.vector.scalar_tensor_tensor(
                        out=tmp["c2"][:], in0=b2[:], scalar=T22SQ,
                        in1=a2[:], op0=Alu.mult, op1=Alu.is_gt)
                    ab = sqpool.tile([n, W], f32, tag="mag0")
                    nc.gpsimd.tensor_tensor(out=ab[:], in0=gxs[0:n, :],
                                            in1=gys[0:n, :], op=Alu.mult)
                    nc.vector.tensor_scalar(out=tmp["sm"][:], in0=ab[:],
                                            scalar1=0.0, scalar2=None,
                                            op0=Alu.is_ge)
                    for t in ("c0", "c2", "sm"):
                        nc.sync.dma_start(
                            out=dst_slices(dst[t]),
                            in_=tmp[t][shift:shift + n_thin, :])

                # ---- B8 block first (feeds the collective) ---------------
                gxs8 = gsumpool.tile([84, W], f32, tag="gxs")
                gys8 = gsumpool.tile([84, W], f32, tag="gys")
                mag8 = []
                for c in range(3):
                    xt = xpool.tile([88, W + 4], f32, tag="x")
                    nc.vector.memset(xt[:, 0:2], 0.0)
                    nc.vector.memset(xt[:, W + 2:W + 4], 0.0)
                    nc.sync.dma_start(out=xt[:, 2:W + 2], in_=x8[c])
                    conv_channel(xt, 88, 86, 84, mt["BV8"], mt["B121_8"],
                                 mt["B121N_8"], mt["B10M1_8"],
                                 mt["B10M1X2_8"],
                                 nc.gpsimd if c != 2 else nc.vector,
                                 gm8[0:84, 1:W + 1], mag8, gxs8, gys8, c,
                                 sqy_bias=mt["GYCOR"][0:84, 0:1])
                gys8c = gsumpool.tile([84, W], f32, tag="gys8c", bufs=1)
                nc.scalar.add(out=gys8c[:], in_=gys8[:],
                              add=mt["GYCOR"][0:84, 1:2])
                make_masks(gxs8, gys8c, 84, 3, 81, m8, lambda t: t[:])

                # D maps: realign gm8 rows and compare in 8 directions
                ce8d = msktpool.tile([81, 1026], f32, tag="dn8", bufs=1)
                up8d = msktpool.tile([81, 1026], f32, tag="up8", bufs=1)
                nc.sync.dma_start(out=ce8[:], in_=gm8[3:84, :])
                nc.sync.dma_start(out=ce8d[:], in_=gm8[4:85, :])
                nc.sync.dma_start(out=up8d[:], in_=gm8[2:83, :])
                d8 = msktpool.tile([81, B, 1024], u8, tag="d8", bufs=1)
                for d, (dr, dc) in DELTAS.items():
                    src = {0: ce8, 1: ce8d, -1: up8d}[dr]
                    nc.vector.tensor_tensor(out=d8[:, d, :],
                                            in0=ce8[:, 1:W + 1],
                                            in1=src[:, 1 + dc:W + 1 + dc],
                                            op=Alu.is_gt)
                for d in range(B):
                    nc.sync.dma_start(out=a2a_in[81 * d:81 * (d + 1), :],
                                      in_=d8[:, d, :])
                nc.gpsimd.collective_compute(
                    "AllToAll", Alu.bypass,
                    replica_groups=[list(range(NC))],
                    ins=[a2a_in.opt()], outs=[a2a_out.opt()])

                # ---- main slab: 8 images x 3 channels --------------------
                for j in range(B):
                    nc.vector.memset(gm_all[:, j, 0:3], 0.0)
                    nc.vector.memset(gm_all[:, j, W + 3:W + 6], 0.0)
                    gxs = gsumpool.tile([122, W], f32, tag="gxs")
                    gys = gsumpool.tile([122, W], f32, tag="gys")
                    mag_c = []
                    for c in range(3):
                        xt = xpool.tile([128, W + 4], f32, tag="x")
                        nc.vector.memset(xt[:, 0:2], 0.0)
                        nc.vector.memset(xt[:, W + 2:W + 4], 0.0)
                        nc.sync.dma_start(out=xt[:, 2:W + 2], in_=xm[3 * j + c])
                        hb = nc.gpsimd if c != 2 else nc.vector
                        conv_channel(xt, 128, 124, 122, mt["BV"], mt["B121"],
                                     mt["B121N"], mt["B10M1"], mt["B10M1X2"],
                                     hb, gm_all[0:122, j, 3:W + 3], mag_c,
                                     gxs, gys, c)
                    make_masks(gxs, gys, 122, 1, 120, mco,
                               lambda t, j=j: t[:, j, 1:W + 1])

            # =========== NMS phase =========================================
            with (
                tc.tile_pool(name="cep", bufs=1) as cepool,
                tc.tile_pool(name="cb", bufs=2) as cbpool,
                tc.tile_pool(name="pb", bufs=1) as pbpool,
                tc.tile_pool(name="tail", bufs=2) as tlpool,
                tc.tile_pool(name="otp", bufs=1) as otpool,
                tc.tile_pool(name="psC", bufs=2, space="PSUM") as psC,
            ):
                ce121 = cepool.tile([121, B, 1030], f32, tag="ce121")
                nc.sync.dma_start(out=ce121[:], in_=gm_all[1:122, :, :])

                def nms_tail(b, pcomp, half):
                    """psel/strong/q/mh/mp/ot/out for output image b.
                    pcomp: P composite [120,4,514] covering
                    w in [base-1 .. base+512]."""
                    base = 512 * half
                    psel = tlpool.tile([120, 514], bf16, tag="psel")
                    nc.scalar.copy(out=psel[:], in_=pcomp[:, 3, :])
                    for t, k in (("sm", 1), ("c0", 0), ("c2", 2)):
                        nc.vector.copy_predicated(
                            out=psel[:], mask=mco[t][:, b, base:base + 514],
                            data=pcomp[:, k, :])
                    hi1 = tlpool.tile([120, 514], bf16, tag="hi1", bufs=1)
                    nc.vector.tensor_scalar(
                        out=hi1[:], in0=ce121[0:120, b, base + 2:base + 516],
                        scalar1=HIGH_T, scalar2=None, op0=Alu.is_gt)
                    strong = tlpool.tile([120, 514], bf16, tag="strong")
                    nc.gpsimd.tensor_tensor(out=strong[:], in0=hi1[:],
                                            in1=psel[:], op=Alu.mult)
                    q1 = tlpool.tile([120, 512], bf16, tag="q1", bufs=1)
                    nc.vector.tensor_scalar(
                        out=q1[:], in0=ce121[0:120, b, base + 3:base + 515],
                        scalar1=LOW_T, scalar2=None, op0=Alu.is_ge)
                    q = tlpool.tile([120, 512], bf16, tag="q")
                    nc.gpsimd.tensor_tensor(out=q[:], in0=q1[:],
                                            in1=psel[:, 1:513],
                                            op=Alu.mult)
                    mh = tlpool.tile([120, 512], bf16, tag="mh")
                    nc.vector.tensor_tensor(out=mh[:], in0=strong[:, 0:512],
                                            in1=strong[:, 2:514], op=Alu.add)
                    nc.vector.tensor_tensor(out=mh[:], in0=mh[:],
                                            in1=strong[:, 1:513], op=Alu.add)
                    mp = psC.tile([119, 512], f32, tag="mp")
                    nc.tensor.matmul(out=mp[:], lhsT=mt["BONES"][0:120, 0:119],
                                     rhs=mh[:], start=True, stop=True)
                    ot = otpool.tile([119, 512], f32, tag="ot")
                    nc.vector.scalar_tensor_tensor(
                        out=ot[:], in0=mp[:], scalar=0.5, in1=q[0:119, :],
                        op0=Alu.is_ge, op1=Alu.logical_and)
                    if half == 0:
                        nc.vector.memset(ot[:, 0:1], 0.0)
                    else:
                        nc.vector.memset(ot[:, 511:512], 0.0)
                    nc.sync.dma_start(out=outm[b][:, base:base + 512],
                                      in_=ot[1:119, :])

                # ---- B8 NMS first: inputs (a2a_out, ce8, m8) are ready
                dr8 = cbpool.tile([81, B, 1024], u8, tag="dr8", bufs=1)
                for d in range(B):
                    nc.sync.dma_start(out=dr8[:, d, :],
                                      in_=a2a_out[81 * d:81 * (d + 1), :])
                p8 = cbpool.tile([81, 4, 1024], u8, tag="p8", bufs=1)
                nc.vector.tensor_tensor(out=p8[:], in0=dr8[:, 0:4, :],
                                        in1=dr8[:, 4:8, :],
                                        op=Alu.logical_and)
                psel8 = cbpool.tile([81, 1024], u8, tag="psel8", bufs=1)
                nc.scalar.copy(out=psel8[:], in_=p8[:, 3, :])
                for t, k in (("sm", 1), ("c0", 0), ("c2", 2)):
                    nc.vector.copy_predicated(out=psel8[:], mask=m8[t][:],
                                              data=p8[:, k, :])
                strong8 = cbpool.tile([81, 1026], bf16, tag="strong8", bufs=1)
                nc.vector.memset(strong8[:, 0:1], 0.0)
                nc.vector.memset(strong8[:, W + 1:W + 2], 0.0)
                hi8 = cbpool.tile([81, 1024], bf16, tag="hi8", bufs=1)
                nc.vector.tensor_scalar(
                    out=hi8[:], in0=ce8[:, 1:W + 1], scalar1=HIGH_T,
                    scalar2=None, op0=Alu.is_gt)
                ps8b = cbpool.tile([81, 1024], bf16, tag="ps8b", bufs=1)
                nc.scalar.copy(out=ps8b[:], in_=psel8[:])
                nc.gpsimd.tensor_tensor(out=strong8[:, 1:W + 1], in0=hi8[:],
                                        in1=ps8b[:], op=Alu.mult)
                q8 = cbpool.tile([81, 1024], u8, tag="q8", bufs=1)
                nc.vector.scalar_tensor_tensor(
                    out=q8[:], in0=ce8[:, 1:W + 1], scalar=LOW_T,
                    in1=psel8[:], op0=Alu.is_ge, op1=Alu.logical_and)
                mh8 = cbpool.tile([81, 1024], bf16, tag="mh8", bufs=1)
                nc.vector.tensor_tensor(out=mh8[:], in0=strong8[:, 0:W],
                                        in1=strong8[:, 2:W + 2], op=Alu.add)
                nc.vector.tensor_tensor(out=mh8[:], in0=mh8[:],
                                        in1=strong8[:, 1:W + 1], op=Alu.add)
                mp8 = psC.tile([81, 1024], f32, tag="mp8", bufs=1)
                for lo in (0, 512):
                    nc.tensor.matmul(out=mp8[:, lo:lo + 512],
                                     lhsT=mt["BONES8"][0:81, 0:81],
                                     rhs=mh8[:, lo:lo + 512],
                                     start=True, stop=True)
                ot8 = cbpool.tile([81, 1024], f32, tag="ot8", bufs=1)
                nc.vector.scalar_tensor_tensor(
                    out=ot8[:], in0=mp8[:], scalar=0.5, in1=q8[:],
                    op0=Alu.is_ge, op1=Alu.logical_and)
                nc.vector.memset(ot8[:, 0:1], 0.0)
                nc.vector.memset(ot8[:, W - 1:W], 0.0)
                nc.sync.dma_start(out=out8[:], in_=ot8[1:81, :])

                for half in (0, 1):
                    base = 512 * half
                    for b in (0, 1, 2, 3):
                        dr, dc = DELTAS[b]
                        # C col i <-> w = base-2+i (516 cols); col(w) = 3+w
                        s0 = base + 1
                        if b == 0:
                            cth = cbpool.tile([120, B, 516], bf16, tag="c0t",
                                              bufs=1)
                            nc.vector.tensor_tensor(
                                out=cth[:],
                                in0=ce121[0:120, :, s0:s0 + 516],
                                in1=ce121[0:120, :, s0 + dc:s0 + dc + 516],
                                op=Alu.is_gt)
                            cx_low = cth  # dr=0: same rows for shifted view
                        else:
                            df = cbpool.tile([121, B, 516], f32, tag="df",
                                             bufs=1)
                            nc.gpsimd.tensor_tensor(
                                out=df[:],
                                in0=gm_all[0:121, :, s0:s0 + 516],
                                in1=ce121[0:121, :, s0 + dc:s0 + dc + 516],
                                op=Alu.subtract)
                            cx = cbpool.tile([121, B, 516], bf16, tag="cx",
                                             bufs=2)
                            nc.vector.tensor_scalar(
                                out=cx[:], in0=df[:], scalar1=0.0,
                                scalar2=None, op0=Alu.is_gt)
                            cth = cbpool.tile([120, B, 516], bf16, tag="cth",
                                              bufs=1)
                            nc.sync.dma_start(out=cth[:], in_=cx[1:121, :, :])
                            cx_low = cx  # rows 0:120 = thin p-1 view
                        # P_b: planes k AND k+4 on w [base-1 .. base+512]
                        pb = pbpool.tile([120, 4, 514], bf16, tag="pb")
                        nc.vector.tensor_tensor(
                            out=pb[:], in0=cth[:, 0:4, 1:515],
                            in1=cth[:, 4:8, 1:515], op=Alu.logical_and)
                        nms_tail(b, pb, half)
                        # P_{b+4} = NOT C_b(p-dr, w-dc) pairwise: sum==0
                        ss = 1 - dc
                        sb = pbpool.tile([120, 4, 514], bf16, tag="sb")
                        nc.vector.tensor_tensor(
                            out=sb[:],
                            in0=cx_low[0:120, 0:4, ss:ss + 514],
                            in1=cx_low[0:120, 4:8, ss:ss + 514],
                            op=Alu.add)
                        pb4 = pbpool.tile([120, 4, 514], bf16, tag="pb4")
                        nc.vector.tensor_scalar(
                            out=pb4[:], in0=sb[:], scalar1=0.0, scalar2=None,
                            op0=Alu.is_equal)
                        nms_tail(b + 4, pb4, half)

    _legalize_waits(nc)
    _CACHE["nc"] = nc
    return nc


def _legalize_waits(nc):
    """Hoist embedded waits of multi-wait instructions into NoOps (several
    ISA encodings hold only one embedded sync-wait)."""
    import concourse.mybir as mybir
    n = 0
    for f in nc.m.functions:
        for blk in f.blocks:
            out = []
            for ins in blk.instructions:
                si = ins.sync_info
                if (si is not None and si.on_wait is not None
                        and len(si.on_wait) > 1):
                    for w in si.on_wait:
                        nop = mybir.InstNoOp(
                            name=f"WFIX-{n}", engine=ins.engine,
                            sync_info=mybir.SyncInfo(on_wait=[w],
                                                     on_update=[]))
                        n += 1
                        out.append(nop)
                    ins.sync_info = mybir.SyncInfo(
                        on_wait=[],
                        on_update=list(si.on_update or []))
                out.append(ins)
            blk.instructions = out


def _in_maps(img):
    img = np.ascontiguousarray(img, dtype=np.float32)
    pad = np.zeros((B, 3, 5, W), np.float32)
    imgp = np.concatenate([pad, img], axis=2)  # rows shifted by +5
    maps = []
    for i in range(NC):
        r0 = SLAB * i
        xm_i = imgp[:, :, r0:r0 + 128, :].reshape(B * 3, 128, W)
        x8_i = img[i, :, B8_START - 8:, :]
        m = {"xm": np.ascontiguousarray(xm_i),
             "x8": np.ascontiguousarray(x8_i)}
        m.update(_const_mats(i))
        maps.append(m)
    return maps


def kernel(img, gauss_h=None, gauss_v=None, sobel_h=None, sobel_v=None,
           dir_f=None, connect_f=None, _want_time=False):
    from concourse.bass_utils import run_bass_kernel_spmd
    nc = _build_program()
    maps = _in_maps(np.asarray(img))
    res = run_bass_kernel_spmd(nc, maps, list(range(NC)), trace=_want_time)
    out = np.zeros((B, 1, H, W), np.float32)
    for i in range(NC):
        r = res.results[i]
        out[:, 0, SLAB * i:SLAB * (i + 1), :] = r["outm"]
        out[i, 0, B8_START:, :] = r["out8"]
    if _want_time:
        return out, res
    return out


# revision 33
# speedup vs baseline: 1.2942x; 1.0342x over previous
"""Canny edge detector on 8 Trainium2 NeuronCores (Bass/Tile) — v2.

Sharding: row slabs (see baseline docstring for why: the reference's flat
gather cross-wires images, so every output pixel needs all 8 images' gm).

v2 changes vs baseline:
- Sobel matmuls run in fp32r (4x PE throughput) with exact precision: the
  blurred field is mean-centered (-MU) and split into hi = f32r(bl') and
  lo = bl' - hi; sobel weights are small integers (exact in fp32r), so
  accumulating hi+lo in PSUM reproduces the f32 result to ~1e-6.
- B8 band computed FIRST; only its 8 direction-compare bitmaps (u8) are
  exchanged via ONE AllToAll (663KB vs 5.3MB AllGather+AllToAll in f32),
  fully overlapped with the main conv.
- gm for all 8 images lives in one 3D composite tile -> NMS compares are
  one instruction per (direction, half) over all images.
- Directions 4..7 reuse directions 0..3: C_{b+4}(p) = NOT C_b(p - delta)
  (exact up to f32 ties, which do not occur for this data).
- Elementwise work spread across DVE/Pool/Act per a makespan balance.
"""

import os

os.environ.setdefault("BY_DEFAULT_DISABLE_SUBTILE_DEPS", "1")

import numpy as np

H = 1024
W = 1024
B = 8
NC = 8
SLAB = 118
B8_START = SLAB * NC          # 944
B8_ROWS = H - B8_START        # 80
LOW_T, HIGH_T = 2.5, 5.0
T22SQ = float(np.float32(np.tan(np.pi / 8.0)) ** 2)
MU = 3.0807319                # E[bl] for uniform input; exactness not needed

DELTAS = {0: (0, 1), 1: (1, 1), 2: (1, 0), 3: (1, -1),
          4: (0, -1), 5: (-1, -1), 6: (-1, 0), 7: (-1, 1)}


def _fp32r_round(v):
    u = np.asarray(v, np.float32).reshape(1).view(np.uint32)
    r = ((u >> 12) & 1) + 0x07FF
    return float(((u + r) & ~np.uint32(0xFFF)).view(np.float32)[0])


MU_HI = _fp32r_round(-MU)                       # hi-margin value (= f32r(-MU))
MU_LO = _fp32r_round(np.float32(-MU) - np.float32(MU_HI))


def _gauss5():
    n = np.arange(5, dtype=np.float32) - 2.0
    return np.exp(-0.5 * n * n).astype(np.float32)


def _band(n_in, n_out, offset, taps):
    m_ = np.zeros((n_in, n_out), np.float32)
    for mm in range(n_out):
        for t, w in enumerate(taps):
            k = mm + offset + t
            if 0 <= k < n_in:
                m_[k, mm] = w
    return m_


def _const_mats(core):
    g = _gauss5()
    g0 = float(g[0])
    mats = {}
    mats["BV"] = _band(128, 124, 0, (g0 * g).tolist())
    b121 = _band(124, 122, 0, [1.0, 2.0, 1.0])
    b10m1 = _band(124, 122, 0, [1.0, 0.0, -1.0])
    if core == 0:  # img rows -2,-1 must yield gm=0 (zero-pad semantics)
        b121[:, 0:2] = 0.0
        b10m1[:, 0:2] = 0.0
    mats["B121"] = b121
    mats["B121N"] = -b121
    mats["B10M1"] = b10m1
    mats["B10M1X2"] = 2.0 * b10m1
    bones = _band(120, 119, -1, [1.0, 1.0, 1.0])
    bones[:, 0] = 0.0
    if core == 0:
        bones[:, 1] = 0.0
    mats["BONES"] = bones
    mats["BV8"] = _band(88, 86, 0, (g0 * g).tolist())
    b121_8 = _band(86, 84, 1, [1.0, 2.0, 1.0])
    b10m1_8 = _band(86, 84, 1, [1.0, 0.0, -1.0])
    mats["B121_8"] = b121_8
    mats["B121N_8"] = -b121_8
    mats["B10M1_8"] = b10m1_8
    mats["B10M1X2_8"] = 2.0 * b10m1_8
    bones8 = _band(81, 81, -1, [1.0, 1.0, 1.0])
    bones8[:, 0] = 0.0
    bones8[:, 80] = 0.0
    mats["BONES8"] = bones8
    gycor = np.zeros((84, 2), np.float32)
    gycor[83, 0] = 4.0 * np.float32(MU)   # clipped B10M1_8 col 83: colsum 1
    gycor[83, 1] = 12.0 * np.float32(MU)  # 3 channels summed, for gys8
    mats["GYCOR"] = gycor
    return {k: np.ascontiguousarray(v, np.float32) for k, v in mats.items()}


MAT_SPECS = {
    "GYCOR": [84, 2],
    "BV": [128, 124], "B121": [124, 122], "B121N": [124, 122],
    "B10M1": [124, 122], "B10M1X2": [124, 122], "BONES": [120, 119],
    "BV8": [88, 86], "B121_8": [86, 84], "B121N_8": [86, 84],
    "B10M1_8": [86, 84], "B10M1X2_8": [86, 84], "BONES8": [81, 81],
}
F32R_MATS = ("B121", "B121N", "B10M1", "B10M1X2",
             "B121_8", "B121N_8", "B10M1_8", "B10M1X2_8")
BF16_MATS = ("BONES", "BONES8")

_CACHE = {}


def _build_program():
    if "nc" in _CACHE:
        return _CACHE["nc"]
    import concourse.bass as bass
    import concourse.mybir as mybir
    from concourse.tile import TileContext

    f32 = mybir.dt.float32
    f32r = mybir.dt.float32r
    bf16 = mybir.dt.bfloat16
    u8 = mybir.dt.uint8
    Alu = mybir.AluOpType

    g = _gauss5()
    R10G = float(g[1] / g[0])
    R20G = float(g[2] / g[0])

    nc = bass.Bass()

    xm = nc.declare_dram_parameter("xm", [B * 3, 128, W], f32, isOutput=False)
    x8 = nc.declare_dram_parameter("x8", [3, 88, W], f32, isOutput=False)
    mat_d = {k: nc.declare_dram_parameter(k, v, f32, isOutput=False)
             for k, v in MAT_SPECS.items()}
    outm = nc.declare_dram_parameter("outm", [B, SLAB, W], f32, isOutput=True)
    out8 = nc.declare_dram_parameter("out8", [B8_ROWS, W], f32, isOutput=True)

    with TileContext(nc) as tc:
        with (
            tc.tile_pool(name="consts", bufs=1) as cpool,
            tc.tile_pool(name="gmp", bufs=1) as gmpool,
            tc.tile_pool(name="mskp", bufs=1) as mskpool,
            tc.tile_pool(name="b8p", bufs=1) as b8pool,
            tc.tile_pool(name="dram", bufs=1, space="DRAM") as dpool,
        ):
            # ---- constants ------------------------------------------------
            mt = {}
            for name, shp in MAT_SPECS.items():
                t = cpool.tile(shp, f32, tag=name)
                nc.sync.dma_start(out=t[:], in_=mat_d[name][:])
                if name in F32R_MATS:
                    tr = cpool.tile(shp, f32r, tag=name + "r")
                    nc.scalar.copy(out=tr[:], in_=t[:])
                    mt[name] = tr
                elif name in BF16_MATS:
                    tb = cpool.tile(shp, bf16, tag=name + "b")
                    nc.scalar.copy(out=tb[:], in_=t[:])
                    mt[name] = tb
                else:
                    mt[name] = t
            bias = cpool.tile([128, 1], f32, tag="bias")
            nc.vector.memset(bias[:], -MU)
            muhi_c = cpool.tile([128, 1], f32, tag="muhi_c")
            nc.vector.memset(muhi_c[:], MU_HI)
            mulo_c = cpool.tile([128, 1], f32, tag="mulo_c")
            nc.vector.memset(mulo_c[:], MU_LO)

            # gm composite: [122, 8, 1030] f32; image j plane, data col 3+w,
            # 3 margin cols each side (needed for shifted compare reads)
            gm_all = gmpool.tile([122, B, 1030], f32, tag="gm_all")
            # B8 gm: [85, 1026] f32; row p <-> img 940+p, data col 1+w
            gm8 = b8pool.tile([85, 1026], f32, tag="gm8")
            nc.vector.memset(gm8[:], 0.0)
            # thin-frame mask composites (u8), 1-col margins per plane
            mco = {t: mskpool.tile([120, B, 1026], u8, tag=f"mc_{t}",
                                   name=f"mc_{t}")
                   for t in ("c0", "c2", "sm")}
            for t in ("c0", "c2", "sm"):
                nc.vector.memset(mco[t][:, :, 0:1], 0)
                nc.vector.memset(mco[t][:, :, W + 1:W + 2], 0)
            m8 = {t: b8pool.tile([81, 1024], u8, tag=f"m8_{t}", name=f"m8_{t}")
                  for t in ("c0", "c2", "sm")}
            ce8 = b8pool.tile([81, 1026], f32, tag="ce8")
            a2a_in = dpool.tile([B * 81, W], u8, tag="a2a_in")
            a2a_out = dpool.tile([B * 81, W], u8, tag="a2a_out")

            # =========== conv phase ========================================
            with (
                tc.tile_pool(name="xin", bufs=3) as xpool,
                tc.tile_pool(name="hbt", bufs=2) as hbpool,
                tc.tile_pool(name="blt", bufs=2) as blpool,
                tc.tile_pool(name="sq", bufs=2) as sqpool,
                tc.tile_pool(name="gsum", bufs=2) as gsumpool,
                tc.tile_pool(name="mskt", bufs=2) as msktpool,
                tc.tile_pool(name="psA", bufs=2, space="PSUM") as psA,
                tc.tile_pool(name="psB", bufs=1, space="PSUM") as psB,
            ):
                def conv_channel(xt, n_in, n_bl, n_gxy, bv, b121, b121n,
                                 b10m1, b10m1x2, hb_eng, gm_dst, mag_c,
                                 gxs, gys, c, sqy_bias=None):
                    """One channel: h-blur, fp32 v-blur, split-f32r sobel,
                    magnitude.  gm_dst: AP for this image's gm slice rows
                    [0:n_gxy]; mag_c: list collecting per-channel mag tiles."""
                    # h-blur, symmetric: h2 = t1 + (g1/g0) t2 + (g2/g0) x2
                    # where t1 = x[-2]+x[2], t2 = x[-1]+x[1].  Pool does the
                    # two adds, DVE the two fused madds.
                    h1 = hbpool.tile([n_in, W], f32, tag="h1")
                    h2 = hbpool.tile([n_in, W], f32, tag="h2")
                    t1 = hbpool.tile([n_in, W], f32, tag="t1")
                    nc.gpsimd.tensor_tensor(out=t1[:], in0=xt[:, 0:W],
                                            in1=xt[:, 4:W + 4], op=Alu.add)
                    nc.gpsimd.tensor_tensor(out=h1[:], in0=xt[:, 1:W + 1],
                                            in1=xt[:, 3:W + 3], op=Alu.add)
                    nc.vector.scalar_tensor_tensor(
                        out=h2[:], in0=h1[:], scalar=R10G, in1=t1[:],
                        op0=Alu.mult, op1=Alu.add)
                    nc.vector.scalar_tensor_tensor(
                        out=h2[:], in0=xt[:, 2:W + 2], scalar=R20G,
                        in1=h2[:], op0=Alu.mult, op1=Alu.add)
                    # v-blur: exact fp32 matmul -> PSUM
                    bl = psA.tile([n_bl, W], f32, tag="bl")
                    for lo in (0, 512):
                        nc.tensor.matmul(out=bl[:, lo:lo + 512],
                                         lhsT=bv[0:n_in, 0:n_bl],
                                         rhs=h2[:, lo:lo + 512],
                                         start=True, stop=True)
                    # center + split into f32r hi/lo with -MU margins
                    bhi = blpool.tile([n_bl, W + 2], f32r, tag="bhi")
                    blo = blpool.tile([n_bl, W + 2], f32r, tag="blo")
                    for mcol, dsts in ((muhi_c, bhi), (mulo_c, blo)):
                        for cs in (slice(0, 1), slice(W + 1, W + 2)):
                            nc.vector.tensor_scalar(
                                out=dsts[:, cs], in0=mcol[0:n_bl, 0:1],
                                scalar1=1.0, scalar2=None, op0=Alu.mult)
                    nc.scalar.add(out=bhi[:, 1:W + 1], in_=bl[:],
                                  add=bias[0:n_bl, 0:1])
                    nc.vector.scalar_tensor_tensor(
                        out=blo[:, 1:W + 1], in0=bl[:], scalar=-MU,
                        in1=bhi[:, 1:W + 1], op0=Alu.add, op1=Alu.subtract)
                    # sobel: 20 f32r matmuls (hi+lo accumulate in PSUM)
                    gx = psB.tile([n_gxy, W], f32, tag="gx")
                    gy = psB.tile([n_gxy, W], f32, tag="gy")
                    for lo in (0, 512):
                        first = True
                        for src in (bhi, blo):
                            sm_ = src[:, 0 + lo:512 + lo]
                            sc = src[:, 1 + lo:513 + lo]
                            sp = src[:, 2 + lo:514 + lo]
                            nc.tensor.matmul(out=gx[:, lo:lo + 512],
                                             lhsT=b121[0:n_bl, 0:n_gxy],
                                             rhs=sm_, start=first, stop=False)
                            nc.tensor.matmul(out=gx[:, lo:lo + 512],
                                             lhsT=b121n[0:n_bl, 0:n_gxy],
                                             rhs=sp, start=False,
                                             stop=(src is blo))
                            nc.tensor.matmul(out=gy[:, lo:lo + 512],
                                             lhsT=b10m1[0:n_bl, 0:n_gxy],
                                             rhs=sp, start=first, stop=False)
                            nc.tensor.matmul(out=gy[:, lo:lo + 512],
                                             lhsT=b10m1x2[0:n_bl, 0:n_gxy],
                                             rhs=sc, start=False, stop=False)
                            nc.tensor.matmul(out=gy[:, lo:lo + 512],
                                             lhsT=b10m1[0:n_bl, 0:n_gxy],
                                             rhs=sm_, start=False,
                                             stop=(src is blo))
                            first = False
                    # magnitude: mag_c[c] = sqrt(gx^2 + gy^2)
                    sqx = sqpool.tile([n_gxy, W], f32, tag="sqx")
                    sqy = sqpool.tile([n_gxy, W], f32, tag="sqy")
                    nc.scalar.square(out=sqx[:], in_=gx[:])
                    if sqy_bias is None:
                        nc.scalar.square(out=sqy[:], in_=gy[:])
                    else:
                        nc.scalar.activation(
                            out=sqy[:], in_=gy[:],
                            func=mybir.ActivationFunctionType.Square,
                            bias=sqy_bias)
                    nc.gpsimd.tensor_tensor(out=sqx[:], in0=sqx[:],
                                            in1=sqy[:], op=Alu.add)
                    mg = sqpool.tile([n_gxy, W], f32, tag=f"mag{c}", bufs=1 if c == 2 else None)
                    nc.scalar.sqrt(out=mg[:], in_=sqx[:])
                    mag_c.append(mg)
                    # orientation sums
                    if c == 0:
                        nc.scalar.copy(out=gxs[0:n_gxy, :], in_=gx[:])
                        nc.scalar.copy(out=gys[0:n_gxy, :], in_=gy[:])
                    else:
                        nc.vector.tensor_tensor(out=gxs[0:n_gxy, :],
                                                in0=gxs[0:n_gxy, :],
                                                in1=gx[:], op=Alu.add)
                        nc.vector.tensor_tensor(out=gys[0:n_gxy, :],
                                                in0=gys[0:n_gxy, :],
                                                in1=gy[:], op=Alu.add)
                    if c == 2:
                        # gm = mag0 + mag1 + mag2; single writer of gm slice
                        t01 = sqpool.tile([n_gxy, W], f32, tag="sqy")
                        nc.vector.tensor_tensor(out=t01[:], in0=mag_c[0][:],
                                                in1=mag_c[1][:], op=Alu.add)
                        nc.gpsimd.tensor_tensor(out=gm_dst, in0=t01[:],
                                                in1=mag_c[2][:], op=Alu.add)

                def make_masks(gxs, gys, n, shift, n_thin, dst, dst_slices):
                    """u8 masks at conv frame [0:n]; DMA rows
                    [shift:shift+n_thin] into dst[t] slices."""
                    a2 = sqpool.tile([n, W], f32, tag="sqx")
                    b2 = sqpool.tile([n, W], f32, tag="sqy")
                    nc.scalar.square(out=a2[:, :], in_=gxs[0:n, :])
                    nc.scalar.square(out=b2[:, :], in_=gys[0:n, :])
                    tmp = {t: msktpool.tile([n, W], u8, tag=f"t{t}", name=f"t{t}")
                           for t in ("c0", "c2", "sm")}
                    nc.vector.scalar_tensor_tensor(
                        out=tmp["c0"][:], in0=a2[:], scalar=T22SQ,
                        in1=b2[:], op0=Alu.mult, op1=Alu.is_gt)
                    nc.vector.scalar_tensor_tensor(
                        out=tmp["c2"][:], in0=b2[:], scalar=T22SQ,
                        in1=a2[:], op0=Alu.mult, op1=Alu.is_gt)
                    ab = sqpool.tile([n, W], f32, tag="mag0")
                    nc.gpsimd.tensor_tensor(out=ab[:], in0=gxs[0:n, :],
                                            in1=gys[0:n, :], op=Alu.mult)
                    nc.vector.tensor_scalar(out=tmp["sm"][:], in0=ab[:],
                                            scalar1=0.0, scalar2=None,
                                            op0=Alu.is_ge)
                    for t in ("c0", "c2", "sm"):
                        nc.sync.dma_start(
                            out=dst_slices(dst[t]),
                            in_=tmp[t][shift:shift + n_thin, :])

                # ---- B8 block first (feeds the collective) ---------------
                gxs8 = gsumpool.tile([84, W], f32, tag="gxs")
                gys8 = gsumpool.tile([84, W], f32, tag="gys")
                mag8 = []
                for c in range(3):
                    xt = xpool.tile([88, W + 4], f32, tag="x")
                    nc.vector.memset(xt[:, 0:2], 0.0)
                    nc.vector.memset(xt[:, W + 2:W + 4], 0.0)
                    nc.sync.dma_start(out=xt[:, 2:W + 2], in_=x8[c])
                    conv_channel(xt, 88, 86, 84, mt["BV8"], mt["B121_8"],
                                 mt["B121N_8"], mt["B10M1_8"],
                                 mt["B10M1X2_8"],
                                 nc.gpsimd if c != 2 else nc.vector,
                                 gm8[0:84, 1:W + 1], mag8, gxs8, gys8, c,
                                 sqy_bias=mt["GYCOR"][0:84, 0:1])
                gys8c = gsumpool.tile([84, W], f32, tag="gys8c", bufs=1)
                nc.scalar.add(out=gys8c[:], in_=gys8[:],
                              add=mt["GYCOR"][0:84, 1:2])
                make_masks(gxs8, gys8c, 84, 3, 81, m8, lambda t: t[:])

                # D maps: realign gm8 rows and compare in 8 directions
                ce8d = msktpool.tile([81, 1026], f32, tag="dn8", bufs=1)
                up8d = msktpool.tile([81, 1026], f32, tag="up8", bufs=1)
                nc.sync.dma_start(out=ce8[:], in_=gm8[3:84, :])
                nc.sync.dma_start(out=ce8d[:], in_=gm8[4:85, :])
                nc.sync.dma_start(out=up8d[:], in_=gm8[2:83, :])
                d8 = msktpool.tile([81, B, 1024], u8, tag="d8", bufs=1)
                for d, (dr, dc) in DELTAS.items():
                    src = {0: ce8, 1: ce8d, -1: up8d}[dr]
                    nc.vector.tensor_tensor(out=d8[:, d, :],
                                            in0=ce8[:, 1:W + 1],
                                            in1=src[:, 1 + dc:W + 1 + dc],
                                            op=Alu.is_gt)
                for d in range(B):
                    nc.sync.dma_start(out=a2a_in[81 * d:81 * (d + 1), :],
                                      in_=d8[:, d, :])
                nc.gpsimd.collective_compute(
                    "AllToAll", Alu.bypass,
                    replica_groups=[list(range(NC))],
                    ins=[a2a_in.opt()], outs=[a2a_out.opt()])

                # ---- main slab: 8 images x 3 channels --------------------
                for j in range(B):
                    nc.vector.memset(gm_all[:, j, 0:3], 0.0)
                    nc.vector.memset(gm_all[:, j, W + 3:W + 6], 0.0)
                    gxs = gsumpool.tile([122, W], f32, tag="gxs")
                    gys = gsumpool.tile([122, W], f32, tag="gys")
                    mag_c = []
                    for c in range(3):
                        xt = xpool.tile([128, W + 4], f32, tag="x")
                        nc.vector.memset(xt[:, 0:2], 0.0)
                        nc.vector.memset(xt[:, W + 2:W + 4], 0.0)
                        nc.sync.dma_start(out=xt[:, 2:W + 2], in_=xm[3 * j + c])
                        hb = nc.gpsimd if c != 2 else nc.vector
                        conv_channel(xt, 128, 124, 122, mt["BV"], mt["B121"],
                                     mt["B121N"], mt["B10M1"], mt["B10M1X2"],
                                     hb, gm_all[0:122, j, 3:W + 3], mag_c,
                                     gxs, gys, c)
                    make_masks(gxs, gys, 122, 1, 120, mco,
                               lambda t, j=j: t[:, j, 1:W + 1])

            # =========== NMS phase =========================================
            with (
                tc.tile_pool(name="cep", bufs=1) as cepool,
                tc.tile_pool(name="cb", bufs=2) as cbpool,
                tc.tile_pool(name="pb", bufs=1) as pbpool,
                tc.tile_pool(name="tail", bufs=2) as tlpool,
                tc.tile_pool(name="otp", bufs=1) as otpool,
                tc.tile_pool(name="psC", bufs=2, space="PSUM") as psC,
            ):
                ce121 = cepool.tile([121, B, 1030], f32, tag="ce121")
                nc.sync.dma_start(out=ce121[:], in_=gm_all[1:122, :, :])

                def nms_tail(b, pcomp, half):
                    """psel/strong/q/mh/mp/ot/out for output image b.
                    pcomp: P composite [120,4,514] covering
                    w in [base-1 .. base+512]."""
                    base = 512 * half
                    psel = tlpool.tile([120, 514], bf16, tag="psel")
                    nc.scalar.copy(out=psel[:], in_=pcomp[:, 3, :])
                    for t, k in (("sm", 1), ("c0", 0), ("c2", 2)):
                        nc.vector.copy_predicated(
                            out=psel[:], mask=mco[t][:, b, base:base + 514],
                            data=pcomp[:, k, :])
                    hi1 = tlpool.tile([120, 514], bf16, tag="hi1", bufs=1)
                    nc.vector.tensor_scalar(
                        out=hi1[:], in0=ce121[0:120, b, base + 2:base + 516],
                        scalar1=HIGH_T, scalar2=None, op0=Alu.is_gt)
                    strong = tlpool.tile([120, 514], bf16, tag="strong")
                    nc.gpsimd.tensor_tensor(out=strong[:], in0=hi1[:],
                                            in1=psel[:], op=Alu.mult)
                    q1 = tlpool.tile([120, 512], bf16, tag="q1", bufs=1)
                    nc.vector.tensor_scalar(
                        out=q1[:], in0=ce121[0:120, b, base + 3:base + 515],
                        scalar1=LOW_T, scalar2=None, op0=Alu.is_ge)
                    q = tlpool.tile([120, 512], bf16, tag="q")
                    nc.gpsimd.tensor_tensor(out=q[:], in0=q1[:],
                                            in1=psel[:, 1:513],
                                            op=Alu.mult)
                    mh = tlpool.tile([120, 512], bf16, tag="mh")
                    nc.vector.tensor_tensor(out=mh[:], in0=strong[:, 0:512],
                                            in1=strong[:, 2:514], op=Alu.add)
                    nc.vector.tensor_tensor(out=mh[:], in0=mh[:],
                                            in1=strong[:, 1:513], op=Alu.add)
                    mp = psC.tile([119, 512], f32, tag="mp")
                    nc.tensor.matmul(out=mp[:], lhsT=mt["BONES"][0:120, 0:119],
                                     rhs=mh[:], start=True, stop=True)
                    ot = otpool.tile([119, 512], f32, tag="ot")
                    nc.vector.scalar_tensor_tensor(
                        out=ot[:], in0=mp[:], scalar=0.5, in1=q[0:119, :],
                        op0=Alu.is_ge, op1=Alu.logical_and)
                    if half == 0:
                        nc.vector.memset(ot[:, 0:1], 0.0)
                    else:
                        nc.vector.memset(ot[:, 511:512], 0.0)
                    nc.sync.dma_start(out=outm[b][:, base:base + 512],
                                      in_=ot[1:119, :])

                # ---- B8 NMS first: inputs (a2a_out, ce8, m8) are ready
                dr8 = cbpool.tile([81, B, 1024], u8, tag="dr8", bufs=1)
                for d in range(B):
                    nc.sync.dma_start(out=dr8[:, d, :],
                                      in_=a2a_out[81 * d:81 * (d + 1), :])
                p8 = cbpool.tile([81, 4, 1024], u8, tag="p8", bufs=1)
                nc.vector.tensor_tensor(out=p8[:], in0=dr8[:, 0:4, :],
                                        in1=dr8[:, 4:8, :],
                                        op=Alu.logical_and)
                psel8 = cbpool.tile([81, 1024], u8, tag="psel8", bufs=1)
                nc.scalar.copy(out=psel8[:], in_=p8[:, 3, :])
                for t, k in (("sm", 1), ("c0", 0), ("c2", 2)):
                    nc.vector.copy_predicated(out=psel8[:], mask=m8[t][:],
                                              data=p8[:, k, :])
                strong8 = cbpool.tile([81, 1026], bf16, tag="strong8", bufs=1)
                nc.vector.memset(strong8[:, 0:1], 0.0)
                nc.vector.memset(strong8[:, W + 1:W + 2], 0.0)
                hi8 = cbpool.tile([81, 1024], bf16, tag="hi8", bufs=1)
                nc.vector.tensor_scalar(
                    out=hi8[:], in0=ce8[:, 1:W + 1], scalar1=HIGH_T,
                    scalar2=None, op0=Alu.is_gt)
                ps8b = cbpool.tile([81, 1024], bf16, tag="ps8b", bufs=1)
                nc.scalar.copy(out=ps8b[:], in_=psel8[:])
                nc.gpsimd.tensor_tensor(out=strong8[:, 1:W + 1], in0=hi8[:],
                                        in1=ps8b[:], op=Alu.mult)
                q8 = cbpool.tile([81, 1024], u8, tag="q8", bufs=1)
                nc.vector.scalar_tensor_tensor(
                    out=q8[:], in0=ce8[:, 1:W + 1], scalar=LOW_T,
                    in1=psel8[:], op0=Alu.is_ge, op1=Alu.logical_and)
                mh8 = cbpool.tile([81, 1024], bf16, tag="mh8", bufs=1)
                nc.vector.tensor_tensor(out=mh8[:], in0=strong8[:, 0:W],
                                        in1=strong8[:, 2:W + 2], op=Alu.add)
                nc.vector.tensor_tensor(out=mh8[:], in0=mh8[:],
                                        in1=strong8[:, 1:W + 1], op=Alu.add)
                mp8 = psC.tile([81, 1024], f32, tag="mp8", bufs=1)
                for lo in (0, 512):
                    nc.tensor.matmul(out=mp8[:, lo:lo + 512],
                                     lhsT=mt["BONES8"][0:81, 0:81],
                                     rhs=mh8[:, lo:lo + 512],
                                     start=True, stop=True)
                ot8 = cbpool.tile([81, 1024], f32, tag="ot8", bufs=1)
                nc.vector.scalar_tensor_tensor(
                    out=ot8[:], in0=mp8[:], scalar=0.5, in1=q8[:],
                    op0=Alu.is_ge, op1=Alu.logical_and)
                nc.vector.memset(ot8[:, 0:1], 0.0)
                nc.vector.memset(ot8[:, W - 1:W], 0.0)
                nc.sync.dma_start(out=out8[:], in_=ot8[1:81, :])

                for half in (0, 1):
                    base = 512 * half
                    for b in (0, 1, 2, 3):
                        dr, dc = DELTAS[b]
                        # C col i <-> w = base-2+i (516 cols); col(w) = 3+w
                        s0 = base + 1
                        if b == 0:
                            cth = cbpool.tile([120, B, 516], bf16, tag="c0t",
                                              bufs=1)
                            nc.vector.tensor_tensor(
                                out=cth[:],
                                in0=ce121[0:120, :, s0:s0 + 516],
                                in1=ce121[0:120, :, s0 + dc:s0 + dc + 516],
                                op=Alu.is_gt)
                            cx_low = cth  # dr=0: same rows for shifted view
                        else:
                            df = cbpool.tile([121, B, 516], f32, tag="df",
                                             bufs=1)
                            nc.gpsimd.tensor_tensor(
                                out=df[:],
                                in0=gm_all[0:121, :, s0:s0 + 516],
                                in1=ce121[0:121, :, s0 + dc:s0 + dc + 516],
                                op=Alu.subtract)
                            cx = cbpool.tile([121, B, 516], bf16, tag="cx",
                                             bufs=2)
                            nc.vector.tensor_scalar(
                                out=cx[:], in0=df[:], scalar1=0.0,
                                scalar2=None, op0=Alu.is_gt)
                            cth = cbpool.tile([120, B, 516], bf16, tag="cth",
                                              bufs=1)
                            nc.sync.dma_start(out=cth[:], in_=cx[1:121, :, :])
                            cx_low = cx  # rows 0:120 = thin p-1 view
                        # P_b: planes k AND k+4 on w [base-1 .. base+512]
                        pb = pbpool.tile([120, 4, 514], bf16, tag="pb")
                        nc.vector.tensor_tensor(
                            out=pb[:], in0=cth[:, 0:4, 1:515],
                            in1=cth[:, 4:8, 1:515], op=Alu.logical_and)
                        nms_tail(b, pb, half)
                        # P_{b+4} = NOT C_b(p-dr, w-dc) pairwise: sum==0
                        ss = 1 - dc
                        sb = pbpool.tile([120, 4, 514], bf16, tag="sb")
                        nc.vector.tensor_tensor(
                            out=sb[:],
                            in0=cx_low[0:120, 0:4, ss:ss + 514],
                            in1=cx_low[0:120, 4:8, ss:ss + 514],
                            op=Alu.add)
                        pb4 = pbpool.tile([120, 4, 514], bf16, tag="pb4")
                        nc.vector.tensor_scalar(
                            out=pb4[:], in0=sb[:], scalar1=0.0, scalar2=None,
                            op0=Alu.is_equal)
                        nms_tail(b + 4, pb4, half)

    _legalize_waits(nc)
    _CACHE["nc"] = nc
    return nc


def _legalize_waits(nc):
    """Hoist embedded waits of multi-wait instructions into NoOps (several
    ISA encodings hold only one embedded sync-wait)."""
    import concourse.mybir as mybir
    n = 0
    for f in nc.m.functions:
        for blk in f.blocks:
            out = []
            for ins in blk.instructions:
                si = ins.sync_info
                if (si is not None and si.on_wait is not None
                        and len(si.on_wait) > 1):
                    for w in si.on_wait:
                        nop = mybir.InstNoOp(
                            name=f"WFIX-{n}", engine=ins.engine,
                            sync_info=mybir.SyncInfo(on_wait=[w],
                                                     on_update=[]))
                        n += 1
                        out.append(nop)
                    ins.sync_info = mybir.SyncInfo(
                        on_wait=[],
                        on_update=list(si.on_update or []))
                out.append(ins)
            blk.instructions = out


def _in_maps(img):
    img = np.ascontiguousarray(img, dtype=np.float32)
    pad = np.zeros((B, 3, 5, W), np.float32)
    imgp = np.concatenate([pad, img], axis=2)  # rows shifted by +5
    maps = []
    for i in range(NC):
        r0 = SLAB * i
        xm_i = imgp[:, :, r0:r0 + 128, :].reshape(B * 3, 128, W)
        x8_i = img[i, :, B8_START - 8:, :]
        m = {"xm": np.ascontiguousarray(xm_i),
             "x8": np.ascontiguousarray(x8_i)}
        m.update(_const_mats(i))
        maps.append(m)
    return maps


def kernel(img, gauss_h=None, gauss_v=None, sobel_h=None, sobel_v=None,
           dir_f=None, connect_f=None, _want_time=False):
    from concourse.bass_utils import run_bass_kernel_spmd
    nc = _build_program()
    maps = _in_maps(np.asarray(img))
    res = run_bass_kernel_spmd(nc, maps, list(range(NC)), trace=_want_time)
    out = np.zeros((B, 1, H, W), np.float32)
    for i in range(NC):
        r = res.results[i]
        out[:, 0, SLAB * i:SLAB * (i + 1), :] = r["outm"]
        out[i, 0, B8_START:, :] = r["out8"]
    if _want_time:
        return out, res
    return out
